# revision 2
# baseline (speedup 1.0000x reference)
"""Trainium2 Bass kernel for nn_DRNN (tree double-LSTM decoder + logits/log_softmax).

v2 strategy (vs v1):
  - Pure data parallel: batch B=128 sharded 16 rows/core over 8 cores.
  - Everything stays in LEVEL ORDER on device; the host permutes OUT rows
    back to natural (b, t) order after the run. This kills the HC/HF DRAM
    round trips, indirect scatters and the 40 pred-head transposes, and lets
    the logits phase start while late tree levels are still running.
  - bf16 state pipeline (h, c, gates add, selections); fp8 logits weights
    kept fully SBUF-resident so the logits loop runs group-outer with the
    log-softmax store pipelined per group (no un-overlapped store tail).
  - Weights are host-pre-transposed to [128, ...] contiguous layouts (single
    big DMAs, no rearrange descriptor storms).
  - pred head computed transposed per level piece via matmuls:
      uvT[q, col] = P_a @ h_a^T (from own-piece transposes)
                  + P_f @ h_f^T (gathered from fraternal round outputs via
                    host-baked 0/1 selection matmuls) ; tanh(+pred_b) -> outT.
"""

import sys

sys.path.insert(0, "/opt/trn_rl_repo")

import numpy as np
import ml_dtypes

import concourse.bass as bass
import concourse.bacc as bacc
import concourse.tile as tile
from concourse import mybir
from concourse import bass_utils
from concourse.masks import make_identity

F32 = mybir.dt.float32
BF16 = mybir.dt.bfloat16
F8 = mybir.dt.float8e4
AF = mybir.ActivationFunctionType
OP = mybir.AluOpType

B, T, E, H, V, FC = 128, 40, 512, 512, 10000, 2048
NC_, BC = 8, 16          # cores, batch per core
G = 4 * H                # 2048 gate dim
NV = 20                  # logits column chunks (matmul)
VC = V // NV             # 500
NST = 8                  # store chunks per group
SC = V // NST            # 1250
NCH = 13                 # fraternal chains per row

BF = ml_dtypes.bfloat16
F8NP = ml_dtypes.float8_e4m3

LAST_RESULTS = None
LAST_EXEC_NS = None
SKIP_LOGITS = False


def _levels(fa):
    L = np.zeros((B, T), dtype=np.int32)
    rows = np.arange(B)
    for i in range(1, T):
        L[:, i] = 1 + L[rows, fa[:, i]]
    return L


def _chunks(n, step=128):
    out = []
    o = 0
    while o < n:
        out.append((o, min(step, n - o)))
        o += step
    return out


def _p4(n):
    return -(-n // 4) * 4


def _build(meta):
    NL, OL, XPAD = meta["NL"], meta["OL"], meta["XPAD"]
    touched = meta["touched"]   # {level: [set(src) per piece]}
    NLV = len(NL)
    NLP = [_p4(n) for n in NL]
    KPREV = [1] + [len(_chunks(NL[l])) for l in range(NLV - 1)]
    NCOL = 16 + XPAD
    NGRP = -(-NCOL // 128)
    # fraternal chunks (208 rows each round)
    FCH = _chunks(BC * NCH)

    nc = bacc.Bacc("TRN2", target_bir_lowering=False, debug=True)

    def din(name, shape, dt):
        return nc.dram_tensor(name, list(shape), dt, kind="ExternalInput")

    emb_a = din("emb_a", [128, 4, XPAD], BF16)
    emb_f = din("emb_f", [128, 4, 512], BF16)
    fcT = din("fcT", [128, 16, BC], BF16)
    fc_wT = din("fc_wT", [128, 16, H], BF16)
    fc_bT = din("fc_bT", [128, 4, 1], F32)
    wih_a = din("wih_a", [128, 4, G], BF16)
    wih_f = din("wih_f", [128, 4, G], BF16)
    whh_a = din("whh_a", [128, 4, G], BF16)
    whh_f = din("whh_f", [128, 4, G], BF16)
    paT = din("paT", [128, 4, H], BF16)
    pfT = din("pfT", [128, 4, H], BF16)
    pred_bT = din("pred_bT", [128, 4, 1], F32)
    lwT = din("lwT", [128, 4, V], F8)
    logit_b = din("logit_b", [1, V], F8)
    bias_a = din("bias_a", [1, G], BF16)
    bias_f = din("bias_f", [1, G], BF16)
    sels = [din(f"sel_{l+1}", [128, KPREV[l], NLP[l]], BF16) for l in range(NLV)]
    selv = [din(f"selv_{l+1}", [128, 5, NLP[l]], BF16) for l in range(NLV)]

    OUT = nc.dram_tensor("OUT", [NGRP * 128, V], F32, kind="ExternalOutput")

    with tile.TileContext(nc) as tc:
        with tc.tile_pool(name="p0", bufs=1) as p0, \
             tc.tile_pool(name="dram", bufs=1, space="DRAM") as pd, \
             tc.tile_pool(name="psg", bufs=4, space="PSUM") as psg, \
             tc.tile_pool(name="pst", bufs=2, space="PSUM") as pst, \
             tc.tile_pool(name="pcg", bufs=1, space="PSUM") as pcg, \
             tc.tile_pool(name="puv", bufs=1, space="PSUM") as puv:

            XF = pd.tile([512, G], BF16)
            XAa = pd.tile([max(NL[0], 1), G], BF16)
            e1 = int(OL[3]) if NLV > 3 else XPAD
            XAb = pd.tile([max(e1 - int(OL[1]), 1), G], BF16)
            XAc = pd.tile([max(XPAD - e1, 4), G], BF16)

            ident = p0.tile([128, 128], BF16)
            make_identity(nc, ident[:])
            ones_bf = p0.tile([1, 128], BF16)
            nc.vector.memset(ones_bf[:], 1.0)

            whh_a_t = p0.tile([128, 4, G], BF16)
            nc.sync.dma_start(whh_a_t[:], whh_a[:])
            paT_t = p0.tile([128, 4, H], BF16)
            pfT_t = p0.tile([128, 4, H], BF16)
            nc.sync.dma_start(paT_t[:], paT[:])
            nc.sync.dma_start(pfT_t[:], pfT[:])
            pred_bT_t = p0.tile([128, 4, 1], F32)
            nc.sync.dma_start(pred_bT_t[:], pred_bT[:])
            sel_ts = []
            selv_ts = []
            for l in range(NLV):
                st_ = p0.tile([128, KPREV[l], NLP[l]], BF16, name=f"sel_t{l}")
                nc.sync.dma_start(st_[:], sels[l][:])
                sel_ts.append(st_)
                sv_ = p0.tile([128, 5, NLP[l]], BF16, name=f"selv_t{l}")
                nc.sync.dma_start(sv_[:], selv[l][:])
                selv_ts.append(sv_)

            lwT_t = p0.tile([128, 4, V], F8)
            nc.sync.dma_start(lwT_t[:], lwT[:])
            lb_t = p0.tile([1, V], F8)
            nc.sync.dma_start(lb_t[:], logit_b[:])
            ones_f8 = p0.tile([1, 128], F8)
            nc.vector.memset(ones_f8[:], 1.0)

            # outT per logits group, fp8, memset for padded columns
            og = []
            for g in range(NGRP):
                t = p0.tile([128, 4, 128], F8, name=f"og{g}")
                nc.vector.memset(t[:], 0.0)
                og.append(t)

            # persistent mid-size state
            xa0T = p0.tile([128, 4, BC], BF16)
            v0 = p0.tile([1, H], BF16)
            vsrc = [p0.tile([128, H], BF16, name=f"vsrc{j}") for j in range(4)]
            hc2_0 = p0.tile([BC, 2 * H], BF16)
            h0T = p0.tile([128, 4, _p4(BC)], BF16)

            def elementwise(gs, c_in, hc2, pc, gact, tc2):
                """gates gs = 4 psum tiles [pc, 512] (i f g o), c_in [pc, H] -> hc2 bf16"""
                nc.scalar.activation(gact[:pc, 0:H], gs[0][:pc, :], AF.Sigmoid)
                nc.scalar.activation(gact[:pc, H:2 * H], gs[1][:pc, :], AF.Sigmoid)
                nc.scalar.activation(gact[:pc, 2 * H:3 * H], gs[2][:pc, :], AF.Tanh)
                nc.scalar.activation(gact[:pc, 3 * H:4 * H], gs[3][:pc, :], AF.Sigmoid)
                if c_in is not None:
                    nc.vector.tensor_tensor(out=hc2[:pc, H:2 * H], in0=gact[:pc, H:2 * H],
                                            in1=c_in[:pc, :], op=OP.mult)
                    nc.vector.tensor_tensor(out=tc2[:pc, :], in0=gact[:pc, 0:H],
                                            in1=gact[:pc, 2 * H:3 * H], op=OP.mult)
                    nc.vector.tensor_tensor(out=hc2[:pc, H:2 * H], in0=hc2[:pc, H:2 * H],
                                            in1=tc2[:pc, :], op=OP.add)
                else:
                    nc.vector.tensor_tensor(out=hc2[:pc, H:2 * H], in0=gact[:pc, 0:H],
                                            in1=gact[:pc, 2 * H:3 * H], op=OP.mult)
                nc.scalar.activation(tc2[:pc, :], hc2[:pc, H:2 * H], AF.Tanh)
                nc.vector.tensor_tensor(out=hc2[:pc, 0:H], in0=gact[:pc, 3 * H:4 * H],
                                        in1=tc2[:pc, :], op=OP.mult)

            # ---- early phase: fc, hf0 const, projections, fraternal (scoped) ----
            with tc.tile_pool(name="pb", bufs=1) as pb, \
                 tc.tile_pool(name="pfr", bufs=2) as pfr:
                whh_f_t = pb.tile([128, 4, G], BF16)
                nc.sync.dma_start(whh_f_t[:], whh_f[:])
                bias_a_t = pb.tile([1, G], BF16)
                nc.sync.dma_start(bias_a_t[:], bias_a[:])
                bias_f_t = pb.tile([1, G], BF16)
                nc.sync.dma_start(bias_f_t[:], bias_f[:])
                cf0_bf = pb.tile([128, H], BF16)
                hf0 = pb.tile([1, H], BF16)
                hf0T = pb.tile([128, 4, 1], BF16)
                w0f = pb.tile([1, G], BF16)
                hs1 = [pb.tile([128, 2 * H], BF16, name=f"hs1_{j}") for j in range(2)]
                hs1T = [pb.tile([128, 4, 128], BF16, name=f"hs1T_{j}") for j in range(2)]

                # fc path: x_a0T = fc_w @ fc_feats.T (+fc_b)
                with tc.tile_pool(name="pfc", bufs=1) as pfc:
                    fcT_t = pfc.tile([128, 16, BC], BF16)
                    fc_wT_t = pfc.tile([128, 16, H], BF16)
                    fc_bT_t = pfc.tile([128, 4, 1], F32)
                    nc.sync.dma_start(fcT_t[:], fcT[:])
                    nc.sync.dma_start(fc_wT_t[:], fc_wT[:])
                    nc.sync.dma_start(fc_bT_t[:], fc_bT[:])
                    for mm in range(4):
                        pp = pst.tile([128, BC], F32, space="PSUM", tag="ptr")
                        for q in range(16):
                            nc.tensor.matmul(pp[:, :], fc_wT_t[:, q, mm * 128:(mm + 1) * 128],
                                             fcT_t[:, q, :], start=(q == 0), stop=(q == 15))
                        nc.scalar.activation(xa0T[:, mm, :], pp[:, :], AF.Identity,
                                             bias=fc_bT_t[:, mm, :])

                    # hf0 = LSTM(0 input, 0 state) from biases only
                    gactc = pfc.tile([128, G], BF16)
                    for n in range(4):
                        pg = psg.tile([128, 512], F32, space="PSUM", tag="pg")
                        nc.tensor.matmul(pg[:, :], ones_bf[:1, :128],
                                         bias_f_t[:1, n * 512:(n + 1) * 512], start=True, stop=True)
                        fn_ = AF.Tanh if n == 2 else AF.Sigmoid
                        nc.scalar.activation(gactc[:, n * 512:(n + 1) * 512], pg[:, :], fn_)
                    nc.vector.tensor_tensor(out=cf0_bf[:, :], in0=gactc[:, 0:H],
                                            in1=gactc[:, 2 * H:3 * H], op=OP.mult)
                    tcf0 = pfc.tile([128, H], BF16)
                    nc.scalar.activation(tcf0[:, :], cf0_bf[:, :], AF.Tanh)
                    nc.vector.tensor_tensor(out=hf0[:1, :], in0=gactc[:1, 3 * H:4 * H],
                                            in1=tcf0[:1, :], op=OP.mult)
                    for q in range(4):
                        pt = pst.tile([128, 128], BF16, space="PSUM", tag="ptr")
                        nc.tensor.transpose(pt[:, :1], hf0[0:1, q * 128:(q + 1) * 128], ident[:1, :1])
                        nc.vector.tensor_copy(hf0T[:, q, :], pt[:, :1])
                    # w0f = hf0 @ whh_f.T ; v0 = hf0 @ pred_w[:, H:].T
                    for n in range(4):
                        pg = psg.tile([128, 512], F32, space="PSUM", tag="pg")
                        for q in range(4):
                            nc.tensor.matmul(pg[:1, :], hf0T[:, q, :],
                                             whh_f_t[:, q, n * 512:(n + 1) * 512],
                                             start=(q == 0), stop=(q == 3))
                        nc.vector.tensor_copy(w0f[:1, n * 512:(n + 1) * 512], pg[:1, :])
                    pg = psg.tile([128, 512], F32, space="PSUM", tag="pg")
                    for q in range(4):
                        nc.tensor.matmul(pg[:1, :], hf0T[:, q, :], pfT_t[:, q, :],
                                         start=(q == 0), stop=(q == 3))
                    nc.vector.tensor_copy(v0[:1, :], pg[:1, :])

                # ---- dense projections (XAa -> XF -> XAb -> XAc) + L0 gates ----
                with tc.tile_pool(name="pproj", bufs=1) as ppj, \
                     tc.tile_pool(name="pw1", bufs=2) as pw1:
                    emb_a_t = ppj.tile([128, 4, XPAD], BF16)
                    emb_f_t = ppj.tile([128, 4, 512], BF16)
                    wih_a_t = ppj.tile([128, 4, G], BF16)
                    wih_f_t = ppj.tile([128, 4, G], BF16)
                    nc.sync.dma_start(emb_a_t[:], emb_a[:])
                    nc.sync.dma_start(emb_f_t[:], emb_f[:])
                    nc.sync.dma_start(wih_a_t[:], wih_a[:])
                    nc.sync.dma_start(wih_f_t[:], wih_f[:])

                    jobs = [(emb_a_t, wih_a_t, bias_a_t, XAa, 0, _chunks(int(OL[1])))]
                    jobs.append((emb_f_t, wih_f_t, bias_f_t, XF, 0, _chunks(512)))
                    jobs.append((emb_a_t, wih_a_t, bias_a_t, XAb, int(OL[1]),
                                 [(int(OL[1]) + o, c) for (o, c) in _chunks(e1 - int(OL[1]))]))
                    jobs.append((emb_a_t, wih_a_t, bias_a_t, XAc, e1,
                                 [(e1 + o, c) for (o, c) in _chunks(XPAD - e1)]))
                    for (src, w, bias_row, dst, base, rows) in jobs:
                        for (ro, rc) in rows:
                            for n in range(4):
                                pg = psg.tile([128, 512], F32, space="PSUM", tag="pg")
                                for q in range(4):
                                    nc.tensor.matmul(pg[:rc, :], src[:, q, ro:ro + rc],
                                                     w[:, q, n * 512:(n + 1) * 512],
                                                     start=(q == 0), stop=False)
                                nc.tensor.matmul(pg[:rc, :], ones_bf[:1, :rc],
                                                 bias_row[:1, n * 512:(n + 1) * 512],
                                                 start=False, stop=True)
                                xc = pw1.tile([128, 512], BF16, tag="xc")
                                if n % 2 == 0:
                                    nc.vector.tensor_copy(xc[:rc, :], pg[:rc, :])
                                else:
                                    nc.scalar.copy(xc[:rc, :], pg[:rc, :])
                                nc.sync.dma_start(dst[ro - base:ro - base + rc,
                                                      n * 512:(n + 1) * 512], xc[:rc, :])

                    # level-0 gates (needs wih_a + xa0T)
                    g0s = []
                    for n in range(4):
                        pg = psg.tile([128, 512], F32, space="PSUM", tag="pg")
                        for q in range(4):
                            nc.tensor.matmul(pg[:BC, :], xa0T[:, q, :],
                                             wih_a_t[:, q, n * 512:(n + 1) * 512],
                                             start=(q == 0), stop=False)
                        nc.tensor.matmul(pg[:BC, :], ones_bf[:1, :BC],
                                         bias_a_t[:1, n * 512:(n + 1) * 512],
                                         start=False, stop=True)
                        g0s.append(pg)
                    gact0 = pw1.tile([BC, G], BF16, tag="gact0")
                    tc20 = pw1.tile([BC, H], BF16, tag="tc20")
                    elementwise(g0s, None, hc2_0, BC, gact0, tc20)
                    nc.vector.memset(h0T[:], 0.0)
                    for q in range(4):
                        pt = pst.tile([128, 128], BF16, space="PSUM", tag="ptr")
                        nc.tensor.transpose(pt[:, :BC], hc2_0[:BC, q * 128:(q + 1) * 128],
                                            ident[:BC, :BC])
                        nc.vector.tensor_copy(h0T[:, q, :BC], pt[:, :BC])

                # ---- fraternal rounds (sibling chains; all inputs precomputed) ----
                def frat_s1(j, o, c):
                    xf_t = pfr.tile([128, G], BF16, tag="xat", name=f"xf1_{j}")
                    nc.sync.dma_start(xf_t[:c, :], XF[o:o + c, :])
                    gs = []
                    for n in range(4):
                        pg = psg.tile([128, 512], F32, space="PSUM", tag="pg")
                        nc.tensor.matmul(pg[:c, :], ones_bf[:1, :c],
                                         w0f[:1, n * 512:(n + 1) * 512], start=True, stop=False)
                        nc.tensor.matmul(pg[:c, :], ident[:c, :c],
                                         xf_t[:c, n * 512:(n + 1) * 512], start=False, stop=True)
                        gs.append(pg)
                    gact = pfr.tile([128, G], BF16, tag="gact")
                    tc2 = pfr.tile([128, H], BF16, tag="tc2")
                    elementwise(gs, cf0_bf, hs1[j], c, gact, tc2)
                    for q in range(4):
                        pt = pst.tile([128, 128], BF16, space="PSUM", tag="ptr")
                        nc.tensor.transpose(pt[:, :c], hs1[j][:c, q * 128:(q + 1) * 128],
                                            ident[:c, :c])
                        nc.vector.tensor_copy(hs1T[j][:, q, :c], pt[:, :c])
                    # v1 = h_s1 @ pred_w[:, H:].T
                    pg = psg.tile([128, 512], F32, space="PSUM", tag="pg")
                    for q in range(4):
                        nc.tensor.matmul(pg[:c, :], hs1T[j][:, q, :c], pfT_t[:, q, :],
                                         start=(q == 0), stop=(q == 3))
                    nc.vector.tensor_copy(vsrc[j][:c, :], pg[:c, :])

                def frat_s2(j, o, c):
                    xf_t = pfr.tile([128, G], BF16, tag="xat", name=f"xf2_{j}")
                    nc.sync.dma_start(xf_t[:c, :], XF[256 + o:256 + o + c, :])
                    gs = []
                    for n in range(4):
                        pg = psg.tile([128, 512], F32, space="PSUM", tag="pg")
                        for q in range(4):
                            nc.tensor.matmul(pg[:c, :], hs1T[j][:, q, :c],
                                             whh_f_t[:, q, n * 512:(n + 1) * 512],
                                             start=(q == 0), stop=False)
                        nc.tensor.matmul(pg[:c, :], ident[:c, :c],
                                         xf_t[:c, n * 512:(n + 1) * 512], start=False, stop=True)
                        gs.append(pg)
                    gact = pfr.tile([128, G], BF16, tag="gact")
                    hc2 = pfr.tile([128, 2 * H], BF16, tag="hs2", name=f"hs2_{j}")
                    tc2 = pfr.tile([128, H], BF16, tag="tc2")
                    elementwise(gs, hs1[j][:, H:2 * H], hc2, c, gact, tc2)
                    hT = pfr.tile([128, 4, 128], BF16, tag="hT", name=f"hs2T_{j}")
                    for q in range(4):
                        pt = pst.tile([128, 128], BF16, space="PSUM", tag="ptr")
                        nc.tensor.transpose(pt[:, :c], hc2[:c, q * 128:(q + 1) * 128],
                                            ident[:c, :c])
                        nc.vector.tensor_copy(hT[:, q, :c], pt[:, :c])
                    pg = psg.tile([128, 512], F32, space="PSUM", tag="pg")
                    for q in range(4):
                        nc.tensor.matmul(pg[:c, :], hT[:, q, :c], pfT_t[:, q, :],
                                         start=(q == 0), stop=(q == 3))
                    nc.vector.tensor_copy(vsrc[2 + j][:c, :], pg[:c, :])

                for j, (o, c) in enumerate(FCH):
                    frat_s1(j, o, c)
                for j, (o, c) in enumerate(FCH):
                    frat_s2(j, o, c)

            # ---------------- recurrence + pred(T) + logits ----------------
            with tc.tile_pool(name="pxa", bufs=3) as pxa, \
                 tc.tile_pool(name="pw2", bufs=2) as pw2, \
                 tc.tile_pool(name="phc", bufs=1) as phc, \
                 tc.tile_pool(name="pht", bufs=2) as pht, \
                 tc.tile_pool(name="plg", bufs=2) as plg, \
                 tc.tile_pool(name="pls", bufs=2) as pls, \
                 tc.tile_pool(name="poc", bufs=2) as poc:

                def uvT_piece(hT, pcp, col0, vT_jobs, name):
                    """pred-head transposed for one piece: cols [col0, col0+pcp) of outT"""
                    pu = puv.tile([128, 4, 128], F32, space="PSUM", tag="puv")
                    for q in range(4):
                        for k in range(4):
                            nc.tensor.matmul(pu[:, q, :pcp], paT_t[:, k, q * 128:(q + 1) * 128],
                                             hT[:, k, :pcp], start=(k == 0), stop=False)
                        for ji, (lhs, rows, rhs) in enumerate(vT_jobs):
                            nc.tensor.matmul(pu[:, q, :pcp], lhs[:rows, q * 128:(q + 1) * 128],
                                             rhs, start=False, stop=(ji == len(vT_jobs) - 1))
                    # tanh + pred_b -> outT group tiles (split at group boundaries)
                    c = col0
                    while c < col0 + pcp:
                        g = c // 128
                        ce = min(col0 + pcp, (g + 1) * 128)
                        for q in range(4):
                            nc.scalar.activation(og[g][:, q, c - g * 128:ce - g * 128],
                                                 pu[:, q, c - col0:ce - col0], AF.Tanh,
                                                 bias=pred_bT_t[:, q, :])
                        c = ce

                # level-0 pred: all 16 nodes use v0
                uvT_piece(h0T, _p4(BC), 0,
                          [(v0, 1, ones_bf[:1, :_p4(BC)])], "uv0")

                # ---------------- logits group machinery ----------------
                def logits_group(g):
                    rows = min(128, NCOL - g * 128)
                    lgs = plg.tile([128, V], BF16, tag="lgs", name=f"lgs_{g}")
                    sums = pls.tile([128, NV], F32, tag="sums", name=f"sums_{g}")
                    for n in range(NV):
                        pg = psg.tile([128, 512], F32, space="PSUM", tag="pg")
                        for q in range(4):
                            nc.tensor.matmul(pg[:, :VC], og[g][:, q, :],
                                             lwT_t[:, q, n * VC:(n + 1) * VC],
                                             start=(q == 0), stop=False)
                        nc.tensor.matmul(pg[:, :VC], ones_f8[:1, :128],
                                         lb_t[:1, n * VC:(n + 1) * VC], start=False, stop=True)
                        nc.vector.tensor_copy(lgs[:, n * VC:(n + 1) * VC], pg[:, :VC])
                        esc = pls.tile([128, VC], BF16, tag="esc")
                        nc.scalar.activation(esc[:, :], pg[:, :VC], AF.Exp,
                                             accum_out=sums[:, n:n + 1])
                    lse = pls.tile([128, 2], F32, tag="lse", name=f"lse_{g}")
                    nc.vector.tensor_reduce(out=lse[:, 0:1], in_=sums[:, :],
                                            axis=mybir.AxisListType.X, op=OP.add)
                    nc.scalar.activation(lse[:, 1:2], lse[:, 0:1], AF.Ln)
                    for s in range(NST):
                        oc = poc.tile([128, SC], F32, tag="oc")
                        nc.vector.tensor_scalar(out=oc[:rows, :],
                                                in0=lgs[:rows, s * SC:(s + 1) * SC],
                                                scalar1=lse[:rows, 1:2], scalar2=None,
                                                op0=OP.subtract)
                        nc.sync.dma_start(OUT[g * 128:g * 128 + rows, s * SC:(s + 1) * SC],
                                          oc[:rows, :])

                # group g is complete once all levels with out-columns < (g+1)*128 ran
                grp_ready_level = []
                for g in range(NGRP):
                    end = (g + 1) * 128
                    lv = 0
                    for l in range(1, NLV + 1):
                        if 16 + int(OL[l - 1]) < end:
                            lv = l
                    grp_ready_level.append(lv)
                done_groups = set()

                # ---------------- ancestral levels ----------------
                prev_pieces = [(hc2_0, BC)]
                for l in range(1, NLV + 1):
                    sel_t = sel_ts[l - 1]
                    sv_t = selv_ts[l - 1]
                    new_pieces = []
                    for pi, (o_lvl, pc) in enumerate(_chunks(NL[l - 1])):
                        po = int(OL[l - 1]) + o_lvl
                        pcp = min(_p4(pc), 128)
                        xa_t = pxa.tile([128, G], BF16, tag="xat")
                        if po < int(OL[1]):
                            nc.sync.dma_start(xa_t[:pc, :], XAa[po:po + pc, :])
                        elif po < e1:
                            nc.sync.dma_start(xa_t[:pc, :],
                                              XAb[po - int(OL[1]):po - int(OL[1]) + pc, :])
                        else:
                            nc.sync.dma_start(xa_t[:pc, :], XAc[po - e1:po - e1 + pc, :])
                        # gather father haT [512, pc] / c [pc, 512] from prev level pieces
                        haT = pw2.tile([128, 4, 128], BF16, tag="haT")
                        for mm in range(4):
                            ph = pst.tile([128, 128], F32, space="PSUM", tag="ptr")
                            for kj, (hrp, pck) in enumerate(prev_pieces):
                                nc.tensor.matmul(ph[:, :pcp], hrp[:pck, mm * 128:(mm + 1) * 128],
                                                 sel_t[:pck, kj, o_lvl:o_lvl + pcp],
                                                 start=(kj == 0), stop=(kj == len(prev_pieces) - 1))
                            nc.vector.tensor_copy(haT[:, mm, :pc], ph[:, :pc])
                        cgp = pcg.tile([128, 512], F32, space="PSUM", tag="cgp")
                        for kj, (hrp, pck) in enumerate(prev_pieces):
                            nc.tensor.matmul(cgp[:pc, :], sel_t[:pck, kj, o_lvl:o_lvl + pc],
                                             hrp[:pck, H:2 * H],
                                             start=(kj == 0), stop=(kj == len(prev_pieces) - 1))
                        c_sb = pw2.tile([128, H], BF16, tag="csb")
                        nc.vector.tensor_copy(c_sb[:pc, :], cgp[:pc, :])
                        # gates = h_f @ whh_a.T + xa (x added in PSUM via identity)
                        gs = []
                        for n in range(4):
                            pg = psg.tile([128, 512], F32, space="PSUM", tag="pg")
                            for mm in range(4):
                                nc.tensor.matmul(pg[:pc, :], haT[:, mm, :pc],
                                                 whh_a_t[:, mm, n * 512:(n + 1) * 512],
                                                 start=(mm == 0), stop=False)
                            nc.tensor.matmul(pg[:pc, :], ident[:pc, :pc],
                                             xa_t[:pc, n * 512:(n + 1) * 512],
                                             start=False, stop=True)
                            gs.append(pg)
                        gact = pw2.tile([128, G], BF16, tag="gact")
                        hc2 = phc.tile([128, 2 * H], BF16, tag=f"hc2_{(l * 2 + pi) % 4}")
                        tc2 = pw2.tile([128, H], BF16, tag="tc2")
                        elementwise(gs, c_sb, hc2, pc, gact, tc2)
                        new_pieces.append((hc2, pc))
                        # own-piece transpose for pred head
                        hT = pht.tile([128, 4, 128], BF16, tag="hT")
                        if pcp > pc:
                            nc.vector.memset(hT[:, :, :], 0.0)
                        for q in range(4):
                            pt = pst.tile([128, 128], BF16, space="PSUM", tag="ptr")
                            nc.tensor.transpose(pt[:, :pc], hc2[:pc, q * 128:(q + 1) * 128],
                                                ident[:pc, :pc])
                            nc.vector.tensor_copy(hT[:, q, :pc], pt[:, :pc])
                        # pred head: uvT = P_a @ h^T + P_f @ h_f^T(gathered)
                        vt_jobs = []
                        srcs = sorted(touched[l][pi])
                        for si, src in enumerate(srcs):
                            if src == 4:
                                vt_jobs.append((v0, 1, sv_t[:1, 4, o_lvl:o_lvl + pcp]))
                            else:
                                rows = 128 if src in (0, 2) else (BC * NCH - 128)
                                vt_jobs.append((vsrc[src], rows,
                                                sv_t[:rows, src, o_lvl:o_lvl + pcp]))
                        uvT_piece(hT, pcp, 16 + po, vt_jobs, f"uv_{l}_{pi}")
                    prev_pieces = new_pieces
                    # fire any logits groups that just became ready
                    if not SKIP_LOGITS:
                        for g in range(NGRP - 1):
                            if grp_ready_level[g] <= l and g not in done_groups:
                                done_groups.add(g)
                                logits_group(g)

                if not SKIP_LOGITS:
                    for g in range(NGRP):
                        if g not in done_groups:
                            logits_group(g)

    nc.finalize()
    return nc


def _prep(word_idx, father_idx, fc_feats, embed, fc_w, fc_b,
          a_wih, a_whh, a_bih, a_bhh, f_wih, f_whh, f_bih, f_bhh,
          pred_w, pred_b, logit_w, logit_b):
    wi = np.asarray(word_idx).astype(np.int64)
    fa = np.asarray(father_idx).astype(np.int64)
    fc_feats = np.asarray(fc_feats, dtype=np.float32)
    embed = np.asarray(embed, dtype=np.float32)
    L = _levels(fa)
    Lmax = int(L.max())
    NL = []
    for l in range(1, Lmax + 1):
        NL.append(max(int((L[c * BC:(c + 1) * BC] == l).sum()) for c in range(NC_)))
    OL = np.concatenate([[0], np.cumsum(NL)]).astype(int)
    XPAD = int(OL[-1])
    NLP = [_p4(n) for n in NL]

    embT = np.ascontiguousarray(embed.T)              # [E, V]

    def wlay(w, k):   # [D, k*128] -> [128, k, D]T layout as [128, k, D]
        return np.ascontiguousarray(
            np.asarray(w, np.float32).T.reshape(k, 128, -1).transpose(1, 0, 2))

    wih_aT = wlay(a_wih, 4).astype(BF)
    wih_fT = wlay(f_wih, 4).astype(BF)
    whh_aT = wlay(a_whh, 4).astype(BF)
    whh_fT = wlay(f_whh, 4).astype(BF)
    fc_wT = wlay(fc_w, 16).astype(BF)
    paT_ = wlay(pred_w[:, :H], 4).astype(BF)
    pfT_ = wlay(pred_w[:, H:], 4).astype(BF)
    lwT_ = wlay(logit_w, 4).astype(F8NP)
    pred_bT_ = np.ascontiguousarray(
        np.asarray(pred_b, np.float32).reshape(4, 128, 1).transpose(1, 0, 2))
    fc_bT_ = np.ascontiguousarray(
        np.asarray(fc_b, np.float32).reshape(4, 128, 1).transpose(1, 0, 2))
    bias_a_ = (np.asarray(a_bih, np.float32) + np.asarray(a_bhh, np.float32)).reshape(1, G).astype(BF)
    bias_f_ = (np.asarray(f_bih, np.float32) + np.asarray(f_bhh, np.float32)).reshape(1, G).astype(BF)
    logit_b_ = np.asarray(logit_b, np.float32).reshape(1, V).astype(F8NP)

    touched = {l: [set() for _ in _chunks(NL[l - 1])] for l in range(1, Lmax + 1)}
    in_maps = []
    perms = []
    for c in range(NC_):
        gb0 = c * BC
        Lc = L[gb0:gb0 + BC]
        emb_a_ = np.zeros((4, 128, XPAD), np.float32)
        sels_ = {}
        selv_ = {}
        perm = np.zeros(BC * T, np.int64)
        perm[np.arange(BC) * T] = np.arange(BC)          # i=0 rows
        pos_prev = {(b, 0): b for b in range(BC)}
        for l in range(1, Lmax + 1):
            nodes = [(b, i) for b in range(BC) for i in range(1, T) if Lc[b, i] == l]
            kprev = 1 if l == 1 else len(_chunks(NL[l - 2]))
            sel = np.zeros((kprev, 128, NLP[l - 1]), np.float32)
            sv = np.zeros((5, 128, NLP[l - 1]), np.float32)
            pos_cur = {}
            for j, (b, i) in enumerate(nodes):
                p = int(OL[l - 1]) + j
                pos_cur[(b, i)] = j
                wa = wi[gb0 + b, fa[gb0 + b, i]]
                emb_a_[:, :, p] = embT[:, wa].reshape(4, 128)
                jp = pos_prev[(b, int(fa[gb0 + b, i]))]
                sel[jp // 128, jp % 128, j] = 1.0
                perm[b * T + i] = 16 + p
                pi = j // 128
                if (i - 1) % 3 == 0:
                    sv[4, 0, j] = 1.0
                    touched[l][pi].add(4)
                elif i % 3 == 2:
                    q = b * NCH + (i - 2) // 3
                    src = 0 if q < 128 else 1
                    sv[src, q % 128, j] = 1.0
                    touched[l][pi].add(src)
                else:
                    q = b * NCH + (i - 3) // 3
                    src = 2 if q < 128 else 3
                    sv[src, q % 128, j] = 1.0
                    touched[l][pi].add(src)
            sels_[f"sel_{l}"] = np.ascontiguousarray(sel.transpose(1, 0, 2)).astype(BF)
            selv_[f"selv_{l}"] = np.ascontiguousarray(sv.transpose(1, 0, 2)).astype(BF)
            pos_prev = pos_cur
        emb_f_ = np.zeros((4, 128, 512), np.float32)
        for b in range(BC):
            for k in range(NCH):
                p = b * NCH + k
                emb_f_[:, :, p] = embT[:, wi[gb0 + b, 3 * k + 1]].reshape(4, 128)
                emb_f_[:, :, 256 + p] = embT[:, wi[gb0 + b, 3 * k + 2]].reshape(4, 128)
        fcT_ = np.ascontiguousarray(
            fc_feats[gb0:gb0 + BC].T.reshape(16, 128, BC).transpose(1, 0, 2)).astype(BF)

        in_maps.append({
            "emb_a": np.ascontiguousarray(emb_a_.transpose(1, 0, 2)).astype(BF),
            "emb_f": np.ascontiguousarray(emb_f_.transpose(1, 0, 2)).astype(BF),
            "fcT": fcT_, "fc_wT": fc_wT, "fc_bT": fc_bT_,
            "wih_a": wih_aT, "wih_f": wih_fT, "whh_a": whh_aT, "whh_f": whh_fT,
            "paT": paT_, "pfT": pfT_, "pred_bT": pred_bT_, "lwT": lwT_,
            "logit_b": logit_b_, "bias_a": bias_a_, "bias_f": bias_f_,
            **sels_, **selv_,
        })
        perms.append(perm)
    meta = {"NL": NL, "OL": OL, "XPAD": XPAD, "touched": touched, "perms": perms}
    return in_maps, meta


def kernel(**inputs):
    global LAST_RESULTS, LAST_EXEC_NS
    in_maps, meta = _prep(**inputs)
    nc = _build(meta)
    res = bass_utils.run_bass_kernel_spmd(nc, in_maps, core_ids=list(range(NC_)))
    LAST_RESULTS = res
    LAST_EXEC_NS = res.exec_time_ns
    outs = [res.results[c]["OUT"][meta["perms"][c]].reshape(BC, T, V) for c in range(NC_)]
    return np.concatenate(outs, axis=0).astype(np.float32)


# ---------------------------------------------------------------------------
# Timing helper (not used by grading): paired-timing estimate, see v1 notes.
def _make_runner(nc, in_maps, n_cores=NC_):
    import jax
    from jax.sharding import Mesh, PartitionSpec, NamedSharding
    from concourse import bass2jax

    bass2jax.install_neuronx_cc_hook()
    if nc.dbg_addr is not None:
        in_maps = [{**m, nc.dbg_addr.name: np.zeros((1, 2), np.uint32)} for m in in_maps]
    partition_name = nc.partition_id_tensor.name if nc.partition_id_tensor else None
    in_names, out_names, out_avals, zero_outs = [], [], [], []
    for alloc in nc.m.functions[0].allocations:
        if not isinstance(alloc, mybir.MemoryLocationSet):
            continue
        name = alloc.memorylocations[0].name
        if alloc.kind == "ExternalInput":
            if name != partition_name:
                in_names.append(name)
        elif alloc.kind == "ExternalOutput":
            out_names.append(name)
            shape = tuple(alloc.tensor_shape)
            dtype = mybir.dt.np(alloc.dtype)
            out_avals.append(jax.core.ShapedArray(shape, dtype))
            zero_outs.append(np.zeros(shape, dtype))
    n_params = len(in_names)
    all_in_names = list(in_names) + list(out_names)
    if partition_name is not None:
        all_in_names.append(partition_name)

    def _body(*args):
        operands = list(args)
        if partition_name is not None:
            operands.append(bass2jax.partition_id_tensor())
        outs = bass2jax._bass_exec_p.bind(
            *operands, out_avals=tuple(out_avals), in_names=tuple(all_in_names),
            out_names=tuple(out_names), lowering_input_output_aliases=(),
            sim_require_finite=True, sim_require_nnan=True, nc=nc)
        return tuple(outs)

    devices = jax.devices()[:n_cores]
    mesh = Mesh(np.asarray(devices), ("core",))
    in_specs = (PartitionSpec("core"),) * (n_params + len(out_names))
    out_specs = (PartitionSpec("core"),) * len(out_names)
    sharded = jax.jit(
        jax.shard_map(_body, mesh=mesh, in_specs=in_specs, out_specs=out_specs,
                      check_vma=False), keep_unused=True)
    concat_in = [np.concatenate([np.asarray(in_maps[c][nm]) for c in range(n_cores)], axis=0)
                 for nm in in_names]
    concat_zeros = [np.zeros((n_cores * z.shape[0], *z.shape[1:]), z.dtype) for z in zero_outs]
    sh = NamedSharding(mesh, PartitionSpec("core"))
    dev_args = [jax.device_put(x, sh) for x in concat_in + concat_zeros]
    return sharded, dev_args


def _trivial_nc():
    nc = bacc.Bacc("TRN2", target_bir_lowering=False, debug=True)
    x = nc.dram_tensor("x", [128, 512], F32, kind="ExternalInput")
    y = nc.dram_tensor("y", [128, 512], F32, kind="ExternalOutput")
    with tile.TileContext(nc) as tc:
        with tc.tile_pool(name="sb", bufs=2) as pool:
            t = pool.tile([128, 512], F32)
            nc.sync.dma_start(t[:], x[:])
            t2 = pool.tile([128, 512], F32)
            nc.scalar.mul(t2[:], t[:], 2.0)
            nc.sync.dma_start(y[:], t2[:])
    nc.finalize()
    im = [{"x": np.zeros((128, 512), np.float32)} for _ in range(NC_)]
    return nc, im


def bench_ns(inputs, pairs=40):
    import time
    import jax
    in_maps, meta = _prep(**inputs)
    nc = _build(meta)
    run_k, args_k = _make_runner(nc, in_maps)
    tnc, tim = _trivial_nc()
    run_t, args_t = _make_runner(tnc, tim)
    jax.block_until_ready(run_k(*args_k))
    jax.block_until_ready(run_t(*args_t))
    dk, dt = [], []
    for _ in range(pairs):
        t0 = time.perf_counter()
        jax.block_until_ready(run_t(*args_t))
        t1 = time.perf_counter()
        jax.block_until_ready(run_k(*args_k))
        t2 = time.perf_counter()
        dt.append(t1 - t0)
        dk.append(t2 - t1)
    dk, dt = np.array(dk), np.array(dt)
    est = np.median(dk) - np.median(dt)
    est_min = dk.min() - dt.min()
    return int(est * 1e9), int(est_min * 1e9)


# revision 4
# speedup vs baseline: 1.1505x; 1.1505x over previous
"""Trainium2 Bass kernel for nn_DRNN (tree double-LSTM decoder + logits/log_softmax).

v2 strategy (vs v1):
  - Pure data parallel: batch B=128 sharded 16 rows/core over 8 cores.
  - Everything stays in LEVEL ORDER on device; the host permutes OUT rows
    back to natural (b, t) order after the run. This kills the HC/HF DRAM
    round trips, indirect scatters and the 40 pred-head transposes, and lets
    the logits phase start while late tree levels are still running.
  - bf16 state pipeline (h, c, gates add, selections); fp8 logits weights
    kept fully SBUF-resident so the logits loop runs group-outer with the
    log-softmax store pipelined per group (no un-overlapped store tail).
  - Weights are host-pre-transposed to [128, ...] contiguous layouts (single
    big DMAs, no rearrange descriptor storms).
  - pred head computed transposed per level piece via matmuls:
      uvT[q, col] = P_a @ h_a^T (from own-piece transposes)
                  + P_f @ h_f^T (gathered from fraternal round outputs via
                    host-baked 0/1 selection matmuls) ; tanh(+pred_b) -> outT.
"""

import sys

sys.path.insert(0, "/opt/trn_rl_repo")

import numpy as np
import ml_dtypes

import concourse.bass as bass
import concourse.bacc as bacc
import concourse.tile as tile
from concourse import mybir
from concourse import bass_utils
from concourse.masks import make_identity

F32 = mybir.dt.float32
BF16 = mybir.dt.bfloat16
F8 = mybir.dt.float8e4
AF = mybir.ActivationFunctionType
OP = mybir.AluOpType

B, T, E, H, V, FC = 128, 40, 512, 512, 10000, 2048
NC_, BC = 8, 16          # cores, batch per core
G = 4 * H                # 2048 gate dim
NV = 20                  # logits column chunks (matmul)
VC = V // NV             # 500
NST = 8                  # store chunks per group
SC = V // NST            # 1250
NCH = 13                 # fraternal chains per row

BF = ml_dtypes.bfloat16
F8NP = ml_dtypes.float8_e4m3

LAST_RESULTS = None
LAST_EXEC_NS = None
SKIP_LOGITS = False


def _levels(fa):
    L = np.zeros((B, T), dtype=np.int32)
    rows = np.arange(B)
    for i in range(1, T):
        L[:, i] = 1 + L[rows, fa[:, i]]
    return L


def _chunks(n, step=128):
    out = []
    o = 0
    while o < n:
        out.append((o, min(step, n - o)))
        o += step
    return out


def _p4(n):
    return -(-n // 4) * 4


def _p16(n):
    return -(-n // 16) * 16


def _build(meta):
    NL, OL, XPAD = meta["NL"], meta["OL"], meta["XPAD"]
    touched = meta["touched"]   # {level: [set(src) per piece]}
    NLV = len(NL)
    NLP = [_p4(n) for n in NL]
    KPREV = [1] + [len(_chunks(NL[l])) for l in range(NLV - 1)]
    NCOL = 16 + XPAD
    NGRP = -(-NCOL // 128)
    # fraternal chunks (208 rows each round)
    FCH = _chunks(BC * NCH)

    nc = bacc.Bacc("TRN2", target_bir_lowering=False, debug=True)

    def din(name, shape, dt):
        return nc.dram_tensor(name, list(shape), dt, kind="ExternalInput")

    emb_a = din("emb_a", [128, 4, _p16(XPAD)], F8)
    emb_f = din("emb_f", [128, 4, 512], F8)
    fcT = din("fcT", [128, 16, BC], BF16)
    fc_wT = din("fc_wT", [128, 16, H], BF16)
    fc_bT = din("fc_bT", [128, 4, 1], F32)
    wih_a = din("wih_a", [128, 4, G], F8)
    wih_f = din("wih_f", [128, 4, G], F8)
    whh_a = din("whh_a", [128, 4, G], F8)
    whh_f = din("whh_f", [128, 4, G], BF16)
    paT = din("paT", [128, 4, H], F8)
    pfT = din("pfT", [128, 4, H], BF16)
    pred_bT = din("pred_bT", [128, 4, 1], F32)
    lwT = din("lwT", [128, 4, V], F8)
    logit_b = din("logit_b", [1, V], F8)
    bias_a = din("bias_a", [1, G], F8)
    bias_f = din("bias_f", [1, G], F8)
    sels = [din(f"sel_{l+1}", [128, KPREV[l], NLP[l]], BF16) for l in range(NLV)]
    selv = [din(f"selv_{l+1}", [128, 5, NLP[l]], F8) for l in range(NLV)]

    OUT = nc.dram_tensor("OUT", [NGRP * 128, V], F32, kind="ExternalOutput")

    with tile.TileContext(nc) as tc:
        with tc.tile_pool(name="p0", bufs=1) as p0, \
             tc.tile_pool(name="dram", bufs=1, space="DRAM") as pd, \
             tc.tile_pool(name="psg", bufs=4, space="PSUM") as psg, \
             tc.tile_pool(name="pst", bufs=2, space="PSUM") as pst, \
             tc.tile_pool(name="pcg", bufs=1, space="PSUM") as pcg, \
             tc.tile_pool(name="puv", bufs=1, space="PSUM") as puv, \
             tc.tile_pool(name="pxa", bufs=3) as pxa, \
             tc.tile_pool(name="pfr", bufs=1) as pfr, \
             tc.tile_pool(name="pw2", bufs=2) as pw2, \
             tc.tile_pool(name="phc", bufs=1) as phc, \
             tc.tile_pool(name="pht", bufs=2) as pht, \
             tc.tile_pool(name="pls", bufs=2) as pls:

            XF = pd.tile([512, G], BF16)
            XAl = [pd.tile([max(NL[l], 4), G], BF16, name=f"XA_{l+1}") for l in range(NLV)]

            ident = p0.tile([128, 128], BF16)
            make_identity(nc, ident[:])
            ones_bf = p0.tile([1, 128], BF16)
            nc.vector.memset(ones_bf[:], 1.0)
            ones_f8 = p0.tile([1, 128], F8)
            nc.vector.memset(ones_f8[:], 1.0)

            paT_t = p0.tile([128, 4, H], F8)
            pfT_t = p0.tile([128, 4, H], BF16)
            nc.sync.dma_start(paT_t[:], paT[:])
            nc.sync.dma_start(pfT_t[:], pfT[:])
            pred_bT_t = p0.tile([128, 4, 1], F32)
            nc.sync.dma_start(pred_bT_t[:], pred_bT[:])
            whh_a_t = p0.tile([128, 4, G], F8)
            whh_f_t = p0.tile([128, 4, G], BF16)
            sel_ts = [p0.tile([128, KPREV[l], NLP[l]], BF16, name=f"sel_t{l}")
                      for l in range(NLV)]
            selv_ts = [p0.tile([128, 5, NLP[l]], F8, name=f"selv_t{l}")
                       for l in range(NLV)]

            og = []
            for g in range(NGRP):
                t = p0.tile([128, 4, 128], F8, name=f"og{g}")
                nc.vector.memset(t[:], 0.0)
                og.append(t)

            xa0T = p0.tile([128, 4, BC], F8)
            cf0_bf = p0.tile([128, H], BF16)
            hf0 = p0.tile([1, H], BF16)
            hf0T = p0.tile([128, 4, 1], BF16)
            w0f = p0.tile([1, G], BF16)
            v0 = p0.tile([1, H], F8)
            vsrc = [p0.tile([128, H], F8, name=f"vsrc{j}") for j in range(4)]
            hc2_0 = p0.tile([BC, 2 * H], BF16)
            h0T = p0.tile([128, 4, _p4(BC)], F8)
            hs1 = [p0.tile([128, 2 * H], BF16, name=f"hs1_{j}") for j in range(2)]
            hs1T = [p0.tile([128, 4, 128], BF16, name=f"hs1T_{j}") for j in range(2)]

            def elementwise(gs, c_in, hc2, pc, gact, tc2):
                """gates gs = 4 psum tiles [pc, 512] (i f g o), c_in [pc, H] -> hc2 bf16"""
                nc.scalar.activation(gact[:pc, 0:H], gs[0][:pc, :], AF.Sigmoid)
                nc.scalar.activation(gact[:pc, H:2 * H], gs[1][:pc, :], AF.Sigmoid)
                nc.scalar.activation(gact[:pc, 2 * H:3 * H], gs[2][:pc, :], AF.Tanh)
                nc.scalar.activation(gact[:pc, 3 * H:4 * H], gs[3][:pc, :], AF.Sigmoid)
                if c_in is not None:
                    nc.vector.tensor_tensor(out=hc2[:pc, H:2 * H], in0=gact[:pc, H:2 * H],
                                            in1=c_in[:pc, :], op=OP.mult)
                    nc.vector.tensor_tensor(out=tc2[:pc, :], in0=gact[:pc, 0:H],
                                            in1=gact[:pc, 2 * H:3 * H], op=OP.mult)
                    nc.vector.tensor_tensor(out=hc2[:pc, H:2 * H], in0=hc2[:pc, H:2 * H],
                                            in1=tc2[:pc, :], op=OP.add)
                else:
                    nc.vector.tensor_tensor(out=hc2[:pc, H:2 * H], in0=gact[:pc, 0:H],
                                            in1=gact[:pc, 2 * H:3 * H], op=OP.mult)
                nc.scalar.activation(tc2[:pc, :], hc2[:pc, H:2 * H], AF.Tanh)
                nc.vector.tensor_tensor(out=hc2[:pc, 0:H], in0=gact[:pc, 3 * H:4 * H],
                                        in1=tc2[:pc, :], op=OP.mult)

            # ---- early scope: fc, hf0 const, projections, L0 (space reused later) ----
            with tc.tile_pool(name="pe1", bufs=1) as pe1, \
                 tc.tile_pool(name="pw1", bufs=2) as pw1, \
                 tc.tile_pool(name="pxc", bufs=8) as pxc:
                fcT_t = pe1.tile([128, 16, BC], BF16)
                fc_wT_t = pe1.tile([128, 16, H], BF16)
                fc_bT_t = pe1.tile([128, 4, 1], F32)
                nc.sync.dma_start(fcT_t[:], fcT[:])
                nc.sync.dma_start(fc_wT_t[:], fc_wT[:])
                nc.sync.dma_start(fc_bT_t[:], fc_bT[:])
                bias_a_t = pe1.tile([1, G], F8)
                nc.sync.dma_start(bias_a_t[:], bias_a[:])
                bias_f_t = pe1.tile([1, G], F8)
                nc.sync.dma_start(bias_f_t[:], bias_f[:])
                emb_a_t = pe1.tile([128, 4, _p16(XPAD)], F8)
                emb_f_t = pe1.tile([128, 4, 512], F8)
                wih_a_t = pe1.tile([128, 4, G], F8)
                wih_f_t = pe1.tile([128, 4, G], F8)
                nc.sync.dma_start(emb_a_t[:], emb_a[:])
                nc.sync.dma_start(wih_a_t[:], wih_a[:])
                nc.sync.dma_start(emb_f_t[:], emb_f[:])
                nc.sync.dma_start(wih_f_t[:], wih_f[:])
                nc.sync.dma_start(whh_f_t[:], whh_f[:])
                nc.sync.dma_start(whh_a_t[:], whh_a[:])
                for l in range(NLV):
                    nc.sync.dma_start(sel_ts[l][:], sels[l][:])
                    nc.sync.dma_start(selv_ts[l][:], selv[l][:])

                # fc path
                for mm in range(4):
                    pp = pst.tile([128, BC], F32, space="PSUM", tag="ptr")
                    for q in range(16):
                        nc.tensor.matmul(pp[:, :], fc_wT_t[:, q, mm * 128:(mm + 1) * 128],
                                         fcT_t[:, q, :], start=(q == 0), stop=(q == 15))
                    nc.scalar.activation(xa0T[:, mm, :], pp[:, :], AF.Identity,
                                         bias=fc_bT_t[:, mm, :])

                # hf0 = LSTM(0 input, 0 state) from biases only
                gactc = pe1.tile([128, G], BF16)
                for n in range(4):
                    pg = psg.tile([128, 512], F32, space="PSUM", tag="pg")
                    nc.tensor.matmul(pg[:, :], ones_f8[:1, :128],
                                     bias_f_t[:1, n * 512:(n + 1) * 512], start=True, stop=True)
                    fn_ = AF.Tanh if n == 2 else AF.Sigmoid
                    nc.scalar.activation(gactc[:, n * 512:(n + 1) * 512], pg[:, :], fn_)
                nc.vector.tensor_tensor(out=cf0_bf[:, :], in0=gactc[:, 0:H],
                                        in1=gactc[:, 2 * H:3 * H], op=OP.mult)
                tcf0 = pe1.tile([128, H], BF16)
                nc.scalar.activation(tcf0[:, :], cf0_bf[:, :], AF.Tanh)
                nc.vector.tensor_tensor(out=hf0[:1, :], in0=gactc[:1, 3 * H:4 * H],
                                        in1=tcf0[:1, :], op=OP.mult)
                for q in range(4):
                    pt = pst.tile([128, 128], BF16, space="PSUM", tag="ptr")
                    nc.tensor.transpose(pt[:, :1], hf0[0:1, q * 128:(q + 1) * 128], ident[:1, :1])
                    nc.vector.tensor_copy(hf0T[:, q, :], pt[:, :1])
                for n in range(4):
                    pg = psg.tile([128, 512], F32, space="PSUM", tag="pg")
                    for q in range(4):
                        nc.tensor.matmul(pg[:1, :], hf0T[:, q, :],
                                         whh_f_t[:, q, n * 512:(n + 1) * 512],
                                         start=(q == 0), stop=(q == 3))
                    nc.vector.tensor_copy(w0f[:1, n * 512:(n + 1) * 512], pg[:1, :])
                pg = psg.tile([128, 512], F32, space="PSUM", tag="pg")
                for q in range(4):
                    nc.tensor.matmul(pg[:1, :], hf0T[:, q, :], pfT_t[:, q, :],
                                     start=(q == 0), stop=(q == 3))
                nc.vector.tensor_copy(v0[:1, :], pg[:1, :])

                def proj(src_t, w, bias_row, dst, base, rows):
                    for (ro, rc) in rows:
                        for n in range(4):
                            pg = psg.tile([128, 512], F32, space="PSUM", tag="pg")
                            for q2 in range(2):
                                nc.tensor.matmul(pg[:rc, :],
                                                 src_t[:, 2 * q2:2 * q2 + 2, ro:ro + rc],
                                                 w[:, 2 * q2:2 * q2 + 2, n * 512:(n + 1) * 512],
                                                 start=(q2 == 0), stop=False,
                                                 perf_mode=mybir.MatmulPerfMode.DoubleRow)
                            nc.tensor.matmul(pg[:rc, :], ones_f8[:1, :rc],
                                             bias_row[:1, n * 512:(n + 1) * 512],
                                             start=False, stop=True)
                            xc = pxc.tile([128, 512], BF16, tag="xc")
                            nc.vector.tensor_copy(xc[:rc, :], pg[:rc, :])
                            nc.sync.dma_start(dst[ro - base:ro - base + rc,
                                                  n * 512:(n + 1) * 512], xc[:rc, :])

                def proj_level(l):
                    proj(emb_a_t, wih_a_t, bias_a_t, XAl[l - 1], int(OL[l - 1]),
                         [(int(OL[l - 1]) + o, c) for (o, c) in _chunks(NL[l - 1])])

                # XA_1 (level 1) first, then L0 gates, XF (fraternal), then the rest
                proj_level(1)

                g0s = []
                for n in range(4):
                    pg = psg.tile([128, 512], F32, space="PSUM", tag="pg")
                    for q2 in range(2):
                        nc.tensor.matmul(pg[:BC, :], xa0T[:, 2 * q2:2 * q2 + 2, :],
                                         wih_a_t[:, 2 * q2:2 * q2 + 2, n * 512:(n + 1) * 512],
                                         start=(q2 == 0), stop=False,
                                         perf_mode=mybir.MatmulPerfMode.DoubleRow)
                    nc.tensor.matmul(pg[:BC, :], ones_f8[:1, :BC],
                                     bias_a_t[:1, n * 512:(n + 1) * 512],
                                     start=False, stop=True)
                    g0s.append(pg)
                gact0 = pw1.tile([BC, G], BF16, tag="gact0")
                tc20 = pw1.tile([BC, H], BF16, tag="tc20")
                elementwise(g0s, None, hc2_0, BC, gact0, tc20)
                nc.vector.memset(h0T[:], 0.0)
                for q in range(4):
                    pt = pst.tile([128, 128], BF16, space="PSUM", tag="ptr")
                    nc.tensor.transpose(pt[:, :BC], hc2_0[:BC, q * 128:(q + 1) * 128],
                                        ident[:BC, :BC])
                    nc.vector.tensor_copy(h0T[:, q, :BC], pt[:, :BC])

                proj(emb_f_t, wih_f_t, bias_f_t, XF, 0, _chunks(512))
                for l in range(2, NLV + 1):
                    proj_level(l)

            # ---- late pool: logits weights + buffers (reuses early-scope space) ----
            with tc.tile_pool(name="plt", bufs=1) as plt, \
                 tc.tile_pool(name="plg", bufs=2) as plg, \
                 tc.tile_pool(name="poc", bufs=2) as poc:
                lwT_t = plt.tile([128, 4, V], F8)
                nc.gpsimd.dma_start(lwT_t[:], lwT[:])
                lb_t = plt.tile([1, V], F8)
                nc.gpsimd.dma_start(lb_t[:], logit_b[:])

                # ---- fraternal rounds ----
                def frat_s1(j, o, c):
                    xf_t = pfr.tile([128, G], BF16, tag="fxat", name=f"xf1_{j}")
                    nc.sync.dma_start(xf_t[:c, :], XF[o:o + c, :])
                    gs = []
                    for n in range(4):
                        pg = psg.tile([128, 512], F32, space="PSUM", tag="pg")
                        nc.tensor.matmul(pg[:c, :], ones_bf[:1, :c],
                                         w0f[:1, n * 512:(n + 1) * 512], start=True, stop=False)
                        nc.tensor.matmul(pg[:c, :], ident[:c, :c],
                                         xf_t[:c, n * 512:(n + 1) * 512], start=False, stop=True)
                        gs.append(pg)
                    gact = pfr.tile([128, G], BF16, tag="fgact")
                    tc2 = pfr.tile([128, H], BF16, tag="ftc2")
                    elementwise(gs, cf0_bf, hs1[j], c, gact, tc2)
                    for q in range(4):
                        pt = pst.tile([128, 128], BF16, space="PSUM", tag="ptr")
                        nc.tensor.transpose(pt[:, :c], hs1[j][:c, q * 128:(q + 1) * 128],
                                            ident[:c, :c])
                        nc.vector.tensor_copy(hs1T[j][:, q, :c], pt[:, :c])
                    pg = psg.tile([128, 512], F32, space="PSUM", tag="pg")
                    for q in range(4):
                        nc.tensor.matmul(pg[:c, :], hs1T[j][:, q, :c], pfT_t[:, q, :],
                                         start=(q == 0), stop=(q == 3))
                    nc.vector.tensor_copy(vsrc[j][:c, :], pg[:c, :])

                def frat_s2(j, o, c):
                    xf_t = pfr.tile([128, G], BF16, tag="fxat", name=f"xf2_{j}")
                    nc.sync.dma_start(xf_t[:c, :], XF[256 + o:256 + o + c, :])
                    gs = []
                    for n in range(4):
                        pg = psg.tile([128, 512], F32, space="PSUM", tag="pg")
                        nc.tensor.matmul(pg[:c, :], ident[:c, :c],
                                         xf_t[:c, n * 512:(n + 1) * 512], start=True, stop=False)
                        for q in range(4):
                            nc.tensor.matmul(pg[:c, :], hs1T[j][:, q, :c],
                                             whh_f_t[:, q, n * 512:(n + 1) * 512],
                                             start=False, stop=(q == 3))
                        gs.append(pg)
                    gact = pfr.tile([128, G], BF16, tag="fgact")
                    hc2 = pfr.tile([128, 2 * H], BF16, tag="hs2", name=f"hs2_{j}")
                    tc2 = pfr.tile([128, H], BF16, tag="ftc2")
                    elementwise(gs, hs1[j][:, H:2 * H], hc2, c, gact, tc2)
                    hT = pfr.tile([128, 4, 128], BF16, tag="fhT", name=f"hs2T_{j}")
                    for q in range(4):
                        pt = pst.tile([128, 128], BF16, space="PSUM", tag="ptr")
                        nc.tensor.transpose(pt[:, :c], hc2[:c, q * 128:(q + 1) * 128],
                                            ident[:c, :c])
                        nc.vector.tensor_copy(hT[:, q, :c], pt[:, :c])
                    pg = psg.tile([128, 512], F32, space="PSUM", tag="pg")
                    for q in range(4):
                        nc.tensor.matmul(pg[:c, :], hT[:, q, :c], pfT_t[:, q, :],
                                         start=(q == 0), stop=(q == 3))
                    nc.vector.tensor_copy(vsrc[2 + j][:c, :], pg[:c, :])

                for j, (o, c) in enumerate(FCH):
                    frat_s1(j, o, c)
                for j, (o, c) in enumerate(FCH):
                    frat_s2(j, o, c)

                def uvT_piece(hT, pcp, col0, vT_jobs, name):
                    """pred-head transposed for one piece: cols [col0, col0+pcp) of outT"""
                    pu = puv.tile([128, 4, 128], F32, space="PSUM", tag="puv")
                    for q in range(4):
                        for k2 in range(2):
                            nc.tensor.matmul(pu[:, q, :pcp],
                                             paT_t[:, 2 * k2:2 * k2 + 2, q * 128:(q + 1) * 128],
                                             hT[:, 2 * k2:2 * k2 + 2, :pcp],
                                             start=(k2 == 0), stop=False,
                                             perf_mode=mybir.MatmulPerfMode.DoubleRow)
                        for ji, (lhs, rows, rhs) in enumerate(vT_jobs):
                            nc.tensor.matmul(pu[:, q, :pcp],
                                             lhs[:rows, q * 128:(q + 1) * 128],
                                             rhs[:, :pcp],
                                             start=False, stop=(ji == len(vT_jobs) - 1))
                    c = col0
                    while c < col0 + pcp:
                        g = c // 128
                        ce = min(col0 + pcp, (g + 1) * 128)
                        for q in range(4):
                            nc.scalar.activation(og[g][:, q, c - g * 128:ce - g * 128],
                                                 pu[:, q, c - col0:ce - col0], AF.Tanh,
                                                 bias=pred_bT_t[:, q, :])
                        c = ce

                # level-0 pred: all 16 nodes use v0
                uvT_piece(h0T, _p4(BC), 0, [(v0, 1, ones_f8[:1, :_p4(BC)])], "uv0")

                def logits_group(g):
                    rows = min(128, NCOL - g * 128)
                    lgs = plg.tile([128, V], BF16, tag="lgs", name=f"lgs_{g}")
                    sums = pls.tile([128, NV], F32, tag="sums", name=f"sums_{g}")
                    for n in range(NV):
                        pg = psg.tile([128, 512], F32, space="PSUM", tag="pg")
                        for q2 in range(2):
                            nc.tensor.matmul(pg[:, :VC], og[g][:, 2 * q2:2 * q2 + 2, :],
                                             lwT_t[:, 2 * q2:2 * q2 + 2, n * VC:(n + 1) * VC],
                                             start=(q2 == 0), stop=False,
                                             perf_mode=mybir.MatmulPerfMode.DoubleRow)
                        nc.tensor.matmul(pg[:, :VC], ones_f8[:1, :128],
                                         lb_t[:1, n * VC:(n + 1) * VC], start=False, stop=True)
                        nc.vector.tensor_copy(lgs[:, n * VC:(n + 1) * VC], pg[:, :VC])
                        esc = pls.tile([128, VC], BF16, tag="esc")
                        nc.scalar.activation(esc[:, :], pg[:, :VC], AF.Exp,
                                             accum_out=sums[:, n:n + 1])
                    lse = pls.tile([128, 2], F32, tag="lse", name=f"lse_{g}")
                    nc.vector.tensor_reduce(out=lse[:, 0:1], in_=sums[:, :],
                                            axis=mybir.AxisListType.X, op=OP.add)
                    nc.scalar.activation(lse[:, 1:2], lse[:, 0:1], AF.Ln)
                    for s in range(NST):
                        oc = poc.tile([128, SC], F32, tag="oc")
                        nc.gpsimd.tensor_scalar(out=oc[:rows, :],
                                                in0=lgs[:rows, s * SC:(s + 1) * SC],
                                                scalar1=lse[:rows, 1:2], scalar2=None,
                                                op0=OP.subtract)
                        nc.sync.dma_start(OUT[g * 128:g * 128 + rows, s * SC:(s + 1) * SC],
                                          oc[:rows, :])

                grp_ready_level = []
                for g in range(NGRP):
                    end = (g + 1) * 128
                    lv = 0
                    for l in range(1, NLV + 1):
                        if 16 + int(OL[l - 1]) < end:
                            lv = l
                    grp_ready_level.append(lv)
                done_groups = set()

                # ---------------- ancestral levels ----------------
                prev_pieces = [(hc2_0, BC)]
                for l in range(1, NLV + 1):
                    sel_t = sel_ts[l - 1]
                    sv_t = selv_ts[l - 1]
                    new_pieces = []
                    for pi, (o_lvl, pc) in enumerate(_chunks(NL[l - 1])):
                        po = int(OL[l - 1]) + o_lvl
                        pcp = min(_p4(pc), 128)
                        xa_t = pxa.tile([128, G], BF16, tag="xat")
                        nc.sync.dma_start(xa_t[:pc, :], XAl[l - 1][o_lvl:o_lvl + pc, :])
                        haT = pw2.tile([128, 4, 128], F8, tag="haT")
                        for mm in range(4):
                            ph = pst.tile([128, 128], F32, space="PSUM", tag="ptr")
                            for kj, (hrp, pck) in enumerate(prev_pieces):
                                nc.tensor.matmul(ph[:, :pcp], hrp[:pck, mm * 128:(mm + 1) * 128],
                                                 sel_t[:pck, kj, o_lvl:o_lvl + pcp],
                                                 start=(kj == 0), stop=(kj == len(prev_pieces) - 1))
                            nc.vector.tensor_copy(haT[:, mm, :pc], ph[:, :pc])
                        cgp = pcg.tile([128, 512], F32, space="PSUM", tag="cgp")
                        for kj, (hrp, pck) in enumerate(prev_pieces):
                            nc.tensor.matmul(cgp[:pc, :], sel_t[:pck, kj, o_lvl:o_lvl + pc],
                                             hrp[:pck, H:2 * H],
                                             start=(kj == 0), stop=(kj == len(prev_pieces) - 1))
                        c_sb = pw2.tile([128, H], BF16, tag="csb")
                        nc.vector.tensor_copy(c_sb[:pc, :], cgp[:pc, :])
                        gs = []
                        for n in range(4):
                            pg = psg.tile([128, 512], F32, space="PSUM", tag="pg")
                            nc.tensor.matmul(pg[:pc, :], ident[:pc, :pc],
                                             xa_t[:pc, n * 512:(n + 1) * 512],
                                             start=True, stop=False)
                            for m2 in range(2):
                                nc.tensor.matmul(pg[:pc, :], haT[:, 2 * m2:2 * m2 + 2, :pc],
                                                 whh_a_t[:, 2 * m2:2 * m2 + 2, n * 512:(n + 1) * 512],
                                                 start=False, stop=(m2 == 1),
                                                 perf_mode=mybir.MatmulPerfMode.DoubleRow)
                            gs.append(pg)
                        gact = pw2.tile([128, G], BF16, tag="gact")
                        hc2 = phc.tile([128, 2 * H], BF16, tag=f"hc2_{(l * 2 + pi) % 4}")
                        tc2 = pw2.tile([128, H], BF16, tag="tc2")
                        elementwise(gs, c_sb, hc2, pc, gact, tc2)
                        new_pieces.append((hc2, pc))
                        hT = pht.tile([128, 4, 128], F8, tag="hT")
                        if pcp > pc:
                            nc.vector.memset(hT[:, :, :], 0.0)
                        for q in range(4):
                            pt = pst.tile([128, 128], BF16, space="PSUM", tag="ptr")
                            nc.tensor.transpose(pt[:, :pc], hc2[:pc, q * 128:(q + 1) * 128],
                                                ident[:pc, :pc])
                            nc.vector.tensor_copy(hT[:, q, :pc], pt[:, :pc])
                        vt_jobs = []
                        srcs = sorted(touched[l][pi])
                        for si, src in enumerate(srcs):
                            if src == 4:
                                vt_jobs.append((v0, 1, sv_t[:1, 4, o_lvl:o_lvl + pcp]))
                            else:
                                rows = 128 if src in (0, 2) else (BC * NCH - 128)
                                vt_jobs.append((vsrc[src], rows,
                                                sv_t[:rows, src, o_lvl:o_lvl + pcp]))
                        uvT_piece(hT, pcp, 16 + po, vt_jobs, f"uv_{l}_{pi}")
                    prev_pieces = new_pieces
                    if not SKIP_LOGITS:
                        for g in range(NGRP - 1):
                            if grp_ready_level[g] <= l and g not in done_groups:
                                done_groups.add(g)
                                logits_group(g)

                if not SKIP_LOGITS:
                    for g in range(NGRP):
                        if g not in done_groups:
                            logits_group(g)

    nc.finalize()
    return nc


def _prep(word_idx, father_idx, fc_feats, embed, fc_w, fc_b,
          a_wih, a_whh, a_bih, a_bhh, f_wih, f_whh, f_bih, f_bhh,
          pred_w, pred_b, logit_w, logit_b):
    wi = np.asarray(word_idx).astype(np.int64)
    fa = np.asarray(father_idx).astype(np.int64)
    fc_feats = np.asarray(fc_feats, dtype=np.float32)
    embed = np.asarray(embed, dtype=np.float32)
    L = _levels(fa)
    Lmax = int(L.max())
    NL = []
    for l in range(1, Lmax + 1):
        NL.append(max(int((L[c * BC:(c + 1) * BC] == l).sum()) for c in range(NC_)))
    OL = np.concatenate([[0], np.cumsum(NL)]).astype(int)
    XPAD = int(OL[-1])
    NLP = [_p4(n) for n in NL]

    embT = np.ascontiguousarray(embed.T)              # [E, V]

    def wlay(w, k):   # [D, k*128] -> [128, k, D]T layout as [128, k, D]
        return np.ascontiguousarray(
            np.asarray(w, np.float32).T.reshape(k, 128, -1).transpose(1, 0, 2))

    wih_aT = wlay(a_wih, 4).astype(F8NP)
    wih_fT = wlay(f_wih, 4).astype(F8NP)
    whh_aT = wlay(a_whh, 4).astype(F8NP)
    whh_fT = wlay(f_whh, 4).astype(BF)
    fc_wT = wlay(fc_w, 16).astype(BF)
    paT_ = wlay(pred_w[:, :H], 4).astype(F8NP)
    pfT_ = wlay(pred_w[:, H:], 4).astype(BF)
    lwT_ = wlay(logit_w, 4).astype(F8NP)
    pred_bT_ = np.ascontiguousarray(
        np.asarray(pred_b, np.float32).reshape(4, 128, 1).transpose(1, 0, 2))
    fc_bT_ = np.ascontiguousarray(
        np.asarray(fc_b, np.float32).reshape(4, 128, 1).transpose(1, 0, 2))
    bias_a_ = (np.asarray(a_bih, np.float32) + np.asarray(a_bhh, np.float32)).reshape(1, G).astype(F8NP)
    bias_f_ = (np.asarray(f_bih, np.float32) + np.asarray(f_bhh, np.float32)).reshape(1, G).astype(F8NP)
    logit_b_ = np.asarray(logit_b, np.float32).reshape(1, V).astype(F8NP)

    touched = {l: [set() for _ in _chunks(NL[l - 1])] for l in range(1, Lmax + 1)}
    in_maps = []
    perms = []
    for c in range(NC_):
        gb0 = c * BC
        Lc = L[gb0:gb0 + BC]
        emb_a_ = np.zeros((4, 128, -(-XPAD // 16) * 16), np.float32)
        sels_ = {}
        selv_ = {}
        perm = np.zeros(BC * T, np.int64)
        perm[np.arange(BC) * T] = np.arange(BC)          # i=0 rows
        pos_prev = {(b, 0): b for b in range(BC)}
        for l in range(1, Lmax + 1):
            nodes = [(b, i) for b in range(BC) for i in range(1, T) if Lc[b, i] == l]
            kprev = 1 if l == 1 else len(_chunks(NL[l - 2]))
            sel = np.zeros((kprev, 128, NLP[l - 1]), np.float32)
            sv = np.zeros((5, 128, NLP[l - 1]), np.float32)
            pos_cur = {}
            for j, (b, i) in enumerate(nodes):
                p = int(OL[l - 1]) + j
                pos_cur[(b, i)] = j
                wa = wi[gb0 + b, fa[gb0 + b, i]]
                emb_a_[:, :, p] = embT[:, wa].reshape(4, 128)
                jp = pos_prev[(b, int(fa[gb0 + b, i]))]
                sel[jp // 128, jp % 128, j] = 1.0
                perm[b * T + i] = 16 + p
                pi = j // 128
                if (i - 1) % 3 == 0:
                    sv[4, 0, j] = 1.0
                    touched[l][pi].add(4)
                elif i % 3 == 2:
                    q = b * NCH + (i - 2) // 3
                    src = 0 if q < 128 else 1
                    sv[src, q % 128, j] = 1.0
                    touched[l][pi].add(src)
                else:
                    q = b * NCH + (i - 3) // 3
                    src = 2 if q < 128 else 3
                    sv[src, q % 128, j] = 1.0
                    touched[l][pi].add(src)
            sels_[f"sel_{l}"] = np.ascontiguousarray(sel.transpose(1, 0, 2)).astype(BF)
            selv_[f"selv_{l}"] = np.ascontiguousarray(sv.transpose(1, 0, 2)).astype(F8NP)
            pos_prev = pos_cur
        emb_f_ = np.zeros((4, 128, 512), np.float32)
        for b in range(BC):
            for k in range(NCH):
                p = b * NCH + k
                emb_f_[:, :, p] = embT[:, wi[gb0 + b, 3 * k + 1]].reshape(4, 128)
                emb_f_[:, :, 256 + p] = embT[:, wi[gb0 + b, 3 * k + 2]].reshape(4, 128)
        fcT_ = np.ascontiguousarray(
            fc_feats[gb0:gb0 + BC].T.reshape(16, 128, BC).transpose(1, 0, 2)).astype(BF)

        in_maps.append({
            "emb_a": np.ascontiguousarray(emb_a_.transpose(1, 0, 2)).astype(F8NP),
            "emb_f": np.ascontiguousarray(emb_f_.transpose(1, 0, 2)).astype(F8NP),
            "fcT": fcT_, "fc_wT": fc_wT, "fc_bT": fc_bT_,
            "wih_a": wih_aT, "wih_f": wih_fT, "whh_a": whh_aT, "whh_f": whh_fT,
            "paT": paT_, "pfT": pfT_, "pred_bT": pred_bT_, "lwT": lwT_,
            "logit_b": logit_b_, "bias_a": bias_a_, "bias_f": bias_f_,
            **sels_, **selv_,
        })
        perms.append(perm)
    meta = {"NL": NL, "OL": OL, "XPAD": XPAD, "touched": touched, "perms": perms}
    return in_maps, meta


def kernel(**inputs):
    global LAST_RESULTS, LAST_EXEC_NS
    in_maps, meta = _prep(**inputs)
    nc = _build(meta)
    res = bass_utils.run_bass_kernel_spmd(nc, in_maps, core_ids=list(range(NC_)))
    LAST_RESULTS = res
    LAST_EXEC_NS = res.exec_time_ns
    outs = [res.results[c]["OUT"][meta["perms"][c]].reshape(BC, T, V) for c in range(NC_)]
    return np.concatenate(outs, axis=0).astype(np.float32)


# ---------------------------------------------------------------------------
# Timing helper (not used by grading): paired-timing estimate, see v1 notes.
def _make_runner(nc, in_maps, n_cores=NC_):
    import jax
    from jax.sharding import Mesh, PartitionSpec, NamedSharding
    from concourse import bass2jax

    bass2jax.install_neuronx_cc_hook()
    if nc.dbg_addr is not None:
        in_maps = [{**m, nc.dbg_addr.name: np.zeros((1, 2), np.uint32)} for m in in_maps]
    partition_name = nc.partition_id_tensor.name if nc.partition_id_tensor else None
    in_names, out_names, out_avals, zero_outs = [], [], [], []
    for alloc in nc.m.functions[0].allocations:
        if not isinstance(alloc, mybir.MemoryLocationSet):
            continue
        name = alloc.memorylocations[0].name
        if alloc.kind == "ExternalInput":
            if name != partition_name:
                in_names.append(name)
        elif alloc.kind == "ExternalOutput":
            out_names.append(name)
            shape = tuple(alloc.tensor_shape)
            dtype = mybir.dt.np(alloc.dtype)
            out_avals.append(jax.core.ShapedArray(shape, dtype))
            zero_outs.append(np.zeros(shape, dtype))
    n_params = len(in_names)
    all_in_names = list(in_names) + list(out_names)
    if partition_name is not None:
        all_in_names.append(partition_name)

    def _body(*args):
        operands = list(args)
        if partition_name is not None:
            operands.append(bass2jax.partition_id_tensor())
        outs = bass2jax._bass_exec_p.bind(
            *operands, out_avals=tuple(out_avals), in_names=tuple(all_in_names),
            out_names=tuple(out_names), lowering_input_output_aliases=(),
            sim_require_finite=True, sim_require_nnan=True, nc=nc)
        return tuple(outs)

    devices = jax.devices()[:n_cores]
    mesh = Mesh(np.asarray(devices), ("core",))
    in_specs = (PartitionSpec("core"),) * (n_params + len(out_names))
    out_specs = (PartitionSpec("core"),) * len(out_names)
    sharded = jax.jit(
        jax.shard_map(_body, mesh=mesh, in_specs=in_specs, out_specs=out_specs,
                      check_vma=False), keep_unused=True)
    concat_in = [np.concatenate([np.asarray(in_maps[c][nm]) for c in range(n_cores)], axis=0)
                 for nm in in_names]
    concat_zeros = [np.zeros((n_cores * z.shape[0], *z.shape[1:]), z.dtype) for z in zero_outs]
    sh = NamedSharding(mesh, PartitionSpec("core"))
    dev_args = [jax.device_put(x, sh) for x in concat_in + concat_zeros]
    return sharded, dev_args


def _trivial_nc():
    nc = bacc.Bacc("TRN2", target_bir_lowering=False, debug=True)
    x = nc.dram_tensor("x", [128, 512], F32, kind="ExternalInput")
    y = nc.dram_tensor("y", [128, 512], F32, kind="ExternalOutput")
    with tile.TileContext(nc) as tc:
        with tc.tile_pool(name="sb", bufs=2) as pool:
            t = pool.tile([128, 512], F32)
            nc.sync.dma_start(t[:], x[:])
            t2 = pool.tile([128, 512], F32)
            nc.scalar.mul(t2[:], t[:], 2.0)
            nc.sync.dma_start(y[:], t2[:])
    nc.finalize()
    im = [{"x": np.zeros((128, 512), np.float32)} for _ in range(NC_)]
    return nc, im


def bench_ns(inputs, pairs=40):
    import time
    import jax
    in_maps, meta = _prep(**inputs)
    nc = _build(meta)
    run_k, args_k = _make_runner(nc, in_maps)
    tnc, tim = _trivial_nc()
    run_t, args_t = _make_runner(tnc, tim)
    jax.block_until_ready(run_k(*args_k))
    jax.block_until_ready(run_t(*args_t))
    dk, dt = [], []
    for _ in range(pairs):
        t0 = time.perf_counter()
        jax.block_until_ready(run_t(*args_t))
        t1 = time.perf_counter()
        jax.block_until_ready(run_k(*args_k))
        t2 = time.perf_counter()
        dt.append(t1 - t0)
        dk.append(t2 - t1)
    dk, dt = np.array(dk), np.array(dt)
    est = np.median(dk) - np.median(dt)
    est_min = dk.min() - dt.min()
    return int(est * 1e9), int(est_min * 1e9)


# revision 5
# speedup vs baseline: 1.3805x; 1.1999x over previous
"""Trainium2 Bass kernel for nn_DRNN (tree double-LSTM decoder + logits/log_softmax).

v2 strategy (vs v1):
  - Pure data parallel: batch B=128 sharded 16 rows/core over 8 cores.
  - Everything stays in LEVEL ORDER on device; the host permutes OUT rows
    back to natural (b, t) order after the run. This kills the HC/HF DRAM
    round trips, indirect scatters and the 40 pred-head transposes, and lets
    the logits phase start while late tree levels are still running.
  - bf16 state pipeline (h, c, gates add, selections); fp8 logits weights
    kept fully SBUF-resident so the logits loop runs group-outer with the
    log-softmax store pipelined per group (no un-overlapped store tail).
  - Weights are host-pre-transposed to [128, ...] contiguous layouts (single
    big DMAs, no rearrange descriptor storms).
  - pred head computed transposed per level piece via matmuls:
      uvT[q, col] = P_a @ h_a^T (from own-piece transposes)
                  + P_f @ h_f^T (gathered from fraternal round outputs via
                    host-baked 0/1 selection matmuls) ; tanh(+pred_b) -> outT.
"""

import sys

sys.path.insert(0, "/opt/trn_rl_repo")

import numpy as np
import ml_dtypes

import concourse.bass as bass
import concourse.bacc as bacc
import concourse.tile as tile
from concourse import mybir
from concourse import bass_utils
from concourse.masks import make_identity

F32 = mybir.dt.float32
BF16 = mybir.dt.bfloat16
F8 = mybir.dt.float8e4
AF = mybir.ActivationFunctionType
OP = mybir.AluOpType

B, T, E, H, V, FC = 128, 40, 512, 512, 10000, 2048
NC_, BC = 8, 16          # cores, batch per core
G = 4 * H                # 2048 gate dim
NV = 20                  # logits column chunks (matmul)
VC = V // NV             # 500
NST = 4                  # store chunks per group
SC = V // NST            # 1250
NCH = 13                 # fraternal chains per row

BF = ml_dtypes.bfloat16
F8NP = ml_dtypes.float8_e4m3

LAST_RESULTS = None
LAST_EXEC_NS = None
SKIP_LOGITS = False


def _levels(fa):
    L = np.zeros((B, T), dtype=np.int32)
    rows = np.arange(B)
    for i in range(1, T):
        L[:, i] = 1 + L[rows, fa[:, i]]
    return L


def _chunks(n, step=128):
    out = []
    o = 0
    while o < n:
        out.append((o, min(step, n - o)))
        o += step
    return out


def _p4(n):
    return -(-n // 4) * 4


def _p16(n):
    return -(-n // 16) * 16


def _build(meta):
    NL, OL, XPAD = meta["NL"], meta["OL"], meta["XPAD"]
    touched = meta["touched"]   # {level: [set(src) per piece]}
    NLV = len(NL)
    NLP = [_p4(n) for n in NL]
    KPREV = [1] + [len(_chunks(NL[l])) for l in range(NLV - 1)]
    NCOL = 16 + XPAD
    NGRP = -(-NCOL // 128)
    # fraternal chunks (208 rows each round)
    FCH = _chunks(BC * NCH)

    nc = bacc.Bacc("TRN2", target_bir_lowering=False, debug=True)

    def din(name, shape, dt):
        return nc.dram_tensor(name, list(shape), dt, kind="ExternalInput")

    emb_a = din("emb_a", [128, 4, _p16(XPAD)], F8)
    emb_f = din("emb_f", [128, 4, 512], F8)
    fcT = din("fcT", [128, 16, BC], BF16)
    fc_wT = din("fc_wT", [128, 16, H], BF16)
    fc_bT = din("fc_bT", [128, 4, 1], F32)
    wih_a = din("wih_a", [128, 4, G], F8)
    wih_f = din("wih_f", [128, 4, G], F8)
    whh_a = din("whh_a", [128, 4, G], F8)
    whh_f = din("whh_f", [128, 4, G], BF16)
    paT = din("paT", [128, 4, H], F8)
    pfT = din("pfT", [128, 4, H], BF16)
    pred_bT = din("pred_bT", [128, 4, 1], F32)
    lwT = din("lwT", [128, 4, V], F8)
    logit_b = din("logit_b", [1, V], F8)
    bias_a = din("bias_a", [1, G], F8)
    bias_f = din("bias_f", [1, G], F8)
    sels = [din(f"sel_{l+1}", [128, KPREV[l], NLP[l]], BF16) for l in range(NLV)]
    selv = [din(f"selv_{l+1}", [128, 5, NLP[l]], F8) for l in range(NLV)]

    OUT = nc.dram_tensor("OUT", [NGRP * 128, V], F32, kind="ExternalOutput")

    with tile.TileContext(nc) as tc:
        with tc.tile_pool(name="p0", bufs=1) as p0, \
             tc.tile_pool(name="dram", bufs=1, space="DRAM") as pd, \
             tc.tile_pool(name="psg", bufs=4, space="PSUM") as psg, \
             tc.tile_pool(name="pst", bufs=1, space="PSUM") as pst, \
             tc.tile_pool(name="pcg", bufs=1, space="PSUM") as pcg, \
             tc.tile_pool(name="pgh", bufs=1, space="PSUM") as pgh, \
             tc.tile_pool(name="puv", bufs=1, space="PSUM") as puv, \
             tc.tile_pool(name="pxa", bufs=3) as pxa, \
             tc.tile_pool(name="pfr", bufs=1) as pfr, \
             tc.tile_pool(name="pw2", bufs=2) as pw2, \
             tc.tile_pool(name="phc", bufs=1) as phc, \
             tc.tile_pool(name="pht", bufs=2) as pht, \
             tc.tile_pool(name="pls", bufs=2) as pls:

            XF = pd.tile([512, G], BF16)
            LTAIL = 5                      # levels > LTAIL share one projection tensor
            XAl = [pd.tile([max(NL[l], 4), G], BF16, name=f"XA_{l+1}")
                   for l in range(min(LTAIL, NLV))]
            tail_rows = sum(NL[LTAIL:]) if NLV > LTAIL else 0
            XAt = None
            if tail_rows:
                XAt = pd.tile([max(tail_rows, 4), G], BF16, name="XA_tail")
            tail_off = {}
            off = 0
            for l in range(LTAIL + 1, NLV + 1):
                tail_off[l] = off
                off += NL[l - 1]

            ident = p0.tile([128, 128], BF16)
            make_identity(nc, ident[:])
            ones_bf = p0.tile([1, 128], BF16)
            nc.vector.memset(ones_bf[:], 1.0)
            ones_f8 = p0.tile([1, 128], F8)
            nc.vector.memset(ones_f8[:], 1.0)

            paT_t = p0.tile([128, 4, H], F8)
            pfT_t = p0.tile([128, 4, H], BF16)
            nc.sync.dma_start(paT_t[:], paT[:])
            nc.sync.dma_start(pfT_t[:], pfT[:])
            pred_bT_t = p0.tile([128, 4, 1], F32)
            nc.sync.dma_start(pred_bT_t[:], pred_bT[:])
            whh_a_t = p0.tile([128, 4, G], F8)
            whh_f_t = p0.tile([128, 4, G], BF16)
            sel_ts = [p0.tile([128, KPREV[l], NLP[l]], BF16, name=f"sel_t{l}")
                      for l in range(NLV)]
            selv_ts = [p0.tile([128, 5, NLP[l]], F8, name=f"selv_t{l}")
                       for l in range(NLV)]

            og = []
            for g in range(NGRP):
                t = p0.tile([128, 4, 128], F8, name=f"og{g}")
                nc.vector.memset(t[:], 0.0)
                og.append(t)

            xa0T = p0.tile([128, 4, BC], F8)
            cf0_bf = p0.tile([128, H], BF16)
            hf0 = p0.tile([1, H], BF16)
            hf0T = p0.tile([128, 4, 1], BF16)
            w0f = p0.tile([1, G], BF16)
            v0 = p0.tile([1, H], F8)
            vsrc = [p0.tile([128, H], F8, name=f"vsrc{j}") for j in range(4)]
            hc2_0 = p0.tile([BC, 2 * H], BF16)
            h0T = p0.tile([128, 4, _p4(BC)], F8)
            hs1 = [p0.tile([128, 2 * H], BF16, name=f"hs1_{j}") for j in range(2)]
            hs1T = [p0.tile([128, 4, 128], BF16, name=f"hs1T_{j}") for j in range(2)]

            def elementwise(gs, c_in, hc2, pc, gact, tc2):
                """gates gs = 4 psum tiles [pc, 512] (i f g o), c_in [pc, H] -> hc2 bf16.
                sigmoid(x) = 0.5*tanh(x/2) + 0.5 so only the tanh act table is used;
                the affine is fused into one tensor_scalar per gate on DVE."""
                for k, sc in ((0, 0.5), (1, 0.5), (2, 1.0), (3, 0.5)):
                    nc.scalar.activation(gact[:pc, k * H:(k + 1) * H], gs[k][:pc, :],
                                         AF.Tanh, scale=sc)
                nc.vector.tensor_scalar(out=gact[:pc, 0:2 * H], in0=gact[:pc, 0:2 * H],
                                        scalar1=1.0, scalar2=0.5, op0=OP.add, op1=OP.mult)
                nc.vector.tensor_scalar(out=gact[:pc, 3 * H:4 * H], in0=gact[:pc, 3 * H:4 * H],
                                        scalar1=1.0, scalar2=0.5, op0=OP.add, op1=OP.mult)
                if c_in is not None:
                    nc.vector.tensor_tensor(out=hc2[:pc, H:2 * H], in0=gact[:pc, H:2 * H],
                                            in1=c_in[:pc, :], op=OP.mult)
                    nc.vector.tensor_tensor(out=tc2[:pc, :], in0=gact[:pc, 0:H],
                                            in1=gact[:pc, 2 * H:3 * H], op=OP.mult)
                    nc.vector.tensor_tensor(out=hc2[:pc, H:2 * H], in0=hc2[:pc, H:2 * H],
                                            in1=tc2[:pc, :], op=OP.add)
                else:
                    nc.vector.tensor_tensor(out=hc2[:pc, H:2 * H], in0=gact[:pc, 0:H],
                                            in1=gact[:pc, 2 * H:3 * H], op=OP.mult)
                nc.scalar.activation(tc2[:pc, :], hc2[:pc, H:2 * H], AF.Tanh)
                nc.vector.tensor_tensor(out=hc2[:pc, 0:H], in0=gact[:pc, 3 * H:4 * H],
                                        in1=tc2[:pc, :], op=OP.mult)

            # ---- early scope: fc, hf0 const, projections, L0 (space reused later) ----
            with tc.tile_pool(name="pe1", bufs=1) as pe1, \
                 tc.tile_pool(name="pw1", bufs=2) as pw1, \
                 tc.tile_pool(name="pxc", bufs=8) as pxc:
                fcT_t = pe1.tile([128, 16, BC], BF16)
                fc_wT_t = pe1.tile([128, 16, H], BF16)
                fc_bT_t = pe1.tile([128, 4, 1], F32)
                nc.sync.dma_start(fcT_t[:], fcT[:])
                nc.sync.dma_start(fc_wT_t[:], fc_wT[:])
                nc.sync.dma_start(fc_bT_t[:], fc_bT[:])
                bias_a_t = pe1.tile([1, G], F8)
                nc.sync.dma_start(bias_a_t[:], bias_a[:])
                bias_f_t = pe1.tile([1, G], F8)
                nc.sync.dma_start(bias_f_t[:], bias_f[:])
                emb_a_t = pe1.tile([128, 4, _p16(XPAD)], F8)
                emb_f_t = pe1.tile([128, 4, 512], F8)
                wih_a_t = pe1.tile([128, 4, G], F8)
                wih_f_t = pe1.tile([128, 4, G], F8)
                nc.sync.dma_start(emb_a_t[:], emb_a[:])
                nc.sync.dma_start(wih_a_t[:], wih_a[:])
                nc.sync.dma_start(emb_f_t[:], emb_f[:])
                nc.sync.dma_start(wih_f_t[:], wih_f[:])
                nc.sync.dma_start(whh_f_t[:], whh_f[:])
                nc.sync.dma_start(whh_a_t[:], whh_a[:])
                for l in range(NLV):
                    nc.sync.dma_start(sel_ts[l][:], sels[l][:])
                    nc.sync.dma_start(selv_ts[l][:], selv[l][:])

                # fc path
                for mm in range(4):
                    pp = pst.tile([128, BC], F32, space="PSUM", tag="ptr")
                    for q in range(16):
                        nc.tensor.matmul(pp[:, :], fc_wT_t[:, q, mm * 128:(mm + 1) * 128],
                                         fcT_t[:, q, :], start=(q == 0), stop=(q == 15))
                    nc.scalar.activation(xa0T[:, mm, :], pp[:, :], AF.Identity,
                                         bias=fc_bT_t[:, mm, :])

                # hf0 = LSTM(0 input, 0 state) from biases only
                gactc = pe1.tile([128, G], BF16)
                for n in range(4):
                    pg = psg.tile([128, 512], F32, space="PSUM", tag="pg")
                    nc.tensor.matmul(pg[:, :], ones_f8[:1, :128],
                                     bias_f_t[:1, n * 512:(n + 1) * 512], start=True, stop=True)
                    sc_ = 1.0 if n == 2 else 0.5
                    nc.scalar.activation(gactc[:, n * 512:(n + 1) * 512], pg[:, :],
                                         AF.Tanh, scale=sc_)
                for k in (0, 1, 3):
                    nc.vector.tensor_scalar(out=gactc[:, k * 512:(k + 1) * 512],
                                            in0=gactc[:, k * 512:(k + 1) * 512],
                                            scalar1=1.0, scalar2=0.5,
                                            op0=OP.add, op1=OP.mult)
                nc.vector.tensor_tensor(out=cf0_bf[:, :], in0=gactc[:, 0:H],
                                        in1=gactc[:, 2 * H:3 * H], op=OP.mult)
                tcf0 = pe1.tile([128, H], BF16)
                nc.scalar.activation(tcf0[:, :], cf0_bf[:, :], AF.Tanh)
                nc.vector.tensor_tensor(out=hf0[:1, :], in0=gactc[:1, 3 * H:4 * H],
                                        in1=tcf0[:1, :], op=OP.mult)
                for q in range(4):
                    pt = pst.tile([128, 128], BF16, space="PSUM", tag="ptr")
                    nc.tensor.transpose(pt[:, :1], hf0[0:1, q * 128:(q + 1) * 128], ident[:1, :1])
                    nc.vector.tensor_copy(hf0T[:, q, :], pt[:, :1])
                for n in range(4):
                    pg = psg.tile([128, 512], F32, space="PSUM", tag="pg")
                    for q in range(4):
                        nc.tensor.matmul(pg[:1, :], hf0T[:, q, :],
                                         whh_f_t[:, q, n * 512:(n + 1) * 512],
                                         start=(q == 0), stop=(q == 3))
                    nc.vector.tensor_copy(w0f[:1, n * 512:(n + 1) * 512], pg[:1, :])
                pg = psg.tile([128, 512], F32, space="PSUM", tag="pg")
                for q in range(4):
                    nc.tensor.matmul(pg[:1, :], hf0T[:, q, :], pfT_t[:, q, :],
                                     start=(q == 0), stop=(q == 3))
                nc.vector.tensor_copy(v0[:1, :], pg[:1, :])

                def proj(src_t, w, bias_row, dst, base, rows):
                    for (ro, rc) in rows:
                        for n in range(4):
                            pg = psg.tile([128, 512], F32, space="PSUM", tag="pg")
                            for q2 in range(2):
                                nc.tensor.matmul(pg[:rc, :],
                                                 src_t[:, 2 * q2:2 * q2 + 2, ro:ro + rc],
                                                 w[:, 2 * q2:2 * q2 + 2, n * 512:(n + 1) * 512],
                                                 start=(q2 == 0), stop=False,
                                                 perf_mode=mybir.MatmulPerfMode.DoubleRow)
                            nc.tensor.matmul(pg[:rc, :], ones_f8[:1, :rc],
                                             bias_row[:1, n * 512:(n + 1) * 512],
                                             start=False, stop=True)
                            xc = pxc.tile([128, 512], BF16, tag="xc")
                            if n % 2 == 0:
                                nc.vector.tensor_copy(xc[:rc, :], pg[:rc, :])
                            else:
                                nc.scalar.copy(xc[:rc, :], pg[:rc, :])
                            nc.sync.dma_start(dst[ro - base:ro - base + rc,
                                                  n * 512:(n + 1) * 512], xc[:rc, :])

                def proj_level(l):
                    if l > LTAIL:
                        return
                    proj(emb_a_t, wih_a_t, bias_a_t, XAl[l - 1], int(OL[l - 1]),
                         [(int(OL[l - 1]) + o, c) for (o, c) in _chunks(NL[l - 1])])

                # XA_1 (level 1) first, then L0 gates, XF (fraternal), then the rest
                proj_level(1)

                g0s = []
                for n in range(4):
                    pg = psg.tile([128, 512], F32, space="PSUM", tag="pg")
                    for q2 in range(2):
                        nc.tensor.matmul(pg[:BC, :], xa0T[:, 2 * q2:2 * q2 + 2, :],
                                         wih_a_t[:, 2 * q2:2 * q2 + 2, n * 512:(n + 1) * 512],
                                         start=(q2 == 0), stop=False,
                                         perf_mode=mybir.MatmulPerfMode.DoubleRow)
                    nc.tensor.matmul(pg[:BC, :], ones_f8[:1, :BC],
                                     bias_a_t[:1, n * 512:(n + 1) * 512],
                                     start=False, stop=True)
                    g0s.append(pg)
                gact0 = pw1.tile([BC, G], BF16, tag="gact0")
                tc20 = pw1.tile([BC, H], BF16, tag="tc20")
                elementwise(g0s, None, hc2_0, BC, gact0, tc20)
                nc.vector.memset(h0T[:], 0.0)
                for q in range(4):
                    pt = pst.tile([128, 128], BF16, space="PSUM", tag="ptr")
                    nc.tensor.transpose(pt[:, :BC], hc2_0[:BC, q * 128:(q + 1) * 128],
                                        ident[:BC, :BC])
                    nc.vector.tensor_copy(h0T[:, q, :BC], pt[:, :BC])

                proj(emb_f_t, wih_f_t, bias_f_t, XF, 0, _chunks(512))
                for l in range(2, NLV + 1):
                    proj_level(l)
                if XAt is not None:
                    base = int(OL[LTAIL])
                    proj(emb_a_t, wih_a_t, bias_a_t, XAt, base,
                         [(base + o, c) for (o, c) in _chunks(sum(NL[LTAIL:]))])

            # ---- late pool: logits weights + buffers (reuses early-scope space) ----
            with tc.tile_pool(name="plt", bufs=1) as plt, \
                 tc.tile_pool(name="plg", bufs=2) as plg, \
                 tc.tile_pool(name="poc", bufs=2) as poc:
                lwT_t = plt.tile([128, 4, V], F8)
                nc.gpsimd.dma_start(lwT_t[:], lwT[:])
                lb_t = plt.tile([1, V], F8)
                nc.gpsimd.dma_start(lb_t[:], logit_b[:])

                # ---- fraternal rounds ----
                def frat_s1(j, o, c):
                    xf_t = pfr.tile([128, G], BF16, tag="fxat", name=f"xf1_{j}")
                    nc.sync.dma_start(xf_t[:c, :], XF[o:o + c, :])
                    gs = []
                    for n in range(4):
                        pg = psg.tile([128, 512], F32, space="PSUM", tag="pg")
                        nc.tensor.matmul(pg[:c, :], ones_bf[:1, :c],
                                         w0f[:1, n * 512:(n + 1) * 512], start=True, stop=False)
                        nc.tensor.matmul(pg[:c, :], ident[:c, :c],
                                         xf_t[:c, n * 512:(n + 1) * 512], start=False, stop=True)
                        gs.append(pg)
                    gact = pfr.tile([128, G], BF16, tag="fgact")
                    tc2 = pfr.tile([128, H], BF16, tag="ftc2")
                    elementwise(gs, cf0_bf, hs1[j], c, gact, tc2)
                    for q in range(4):
                        pt = pst.tile([128, 128], BF16, space="PSUM", tag="ptr")
                        nc.tensor.transpose(pt[:, :c], hs1[j][:c, q * 128:(q + 1) * 128],
                                            ident[:c, :c])
                        nc.vector.tensor_copy(hs1T[j][:, q, :c], pt[:, :c])
                    pg = psg.tile([128, 512], F32, space="PSUM", tag="pg")
                    for q in range(4):
                        nc.tensor.matmul(pg[:c, :], hs1T[j][:, q, :c], pfT_t[:, q, :],
                                         start=(q == 0), stop=(q == 3))
                    nc.vector.tensor_copy(vsrc[j][:c, :], pg[:c, :])

                def frat_s2(j, o, c):
                    xf_t = pfr.tile([128, G], BF16, tag="fxat", name=f"xf2_{j}")
                    nc.sync.dma_start(xf_t[:c, :], XF[256 + o:256 + o + c, :])
                    gs = []
                    for n in range(4):
                        pg = psg.tile([128, 512], F32, space="PSUM", tag="pg")
                        nc.tensor.matmul(pg[:c, :], ident[:c, :c],
                                         xf_t[:c, n * 512:(n + 1) * 512], start=True, stop=False)
                        for q in range(4):
                            nc.tensor.matmul(pg[:c, :], hs1T[j][:, q, :c],
                                             whh_f_t[:, q, n * 512:(n + 1) * 512],
                                             start=False, stop=(q == 3))
                        gs.append(pg)
                    gact = pfr.tile([128, G], BF16, tag="fgact")
                    hc2 = pfr.tile([128, 2 * H], BF16, tag="hs2", name=f"hs2_{j}")
                    tc2 = pfr.tile([128, H], BF16, tag="ftc2")
                    elementwise(gs, hs1[j][:, H:2 * H], hc2, c, gact, tc2)
                    hT = pfr.tile([128, 4, 128], BF16, tag="fhT", name=f"hs2T_{j}")
                    for q in range(4):
                        pt = pst.tile([128, 128], BF16, space="PSUM", tag="ptr")
                        nc.tensor.transpose(pt[:, :c], hc2[:c, q * 128:(q + 1) * 128],
                                            ident[:c, :c])
                        nc.vector.tensor_copy(hT[:, q, :c], pt[:, :c])
                    pg = psg.tile([128, 512], F32, space="PSUM", tag="pg")
                    for q in range(4):
                        nc.tensor.matmul(pg[:c, :], hT[:, q, :c], pfT_t[:, q, :],
                                         start=(q == 0), stop=(q == 3))
                    nc.vector.tensor_copy(vsrc[2 + j][:c, :], pg[:c, :])

                for j, (o, c) in enumerate(FCH):
                    frat_s1(j, o, c)
                for j, (o, c) in enumerate(FCH):
                    frat_s2(j, o, c)

                def uvT_piece(hT, pcp, col0, vT_jobs, name):
                    """pred-head transposed for one piece: cols [col0, col0+pcp) of outT"""
                    pu = puv.tile([128, 4, 128], F32, space="PSUM", tag="puv")
                    for q in range(4):
                        for k2 in range(2):
                            nc.tensor.matmul(pu[:, q, :pcp],
                                             paT_t[:, 2 * k2:2 * k2 + 2, q * 128:(q + 1) * 128],
                                             hT[:, 2 * k2:2 * k2 + 2, :pcp],
                                             start=(k2 == 0), stop=False,
                                             perf_mode=mybir.MatmulPerfMode.DoubleRow)
                        for ji, (lhs, rows, rhs) in enumerate(vT_jobs):
                            nc.tensor.matmul(pu[:, q, :pcp],
                                             lhs[:rows, q * 128:(q + 1) * 128],
                                             rhs[:, :pcp],
                                             start=False, stop=(ji == len(vT_jobs) - 1))
                    c = col0
                    while c < col0 + pcp:
                        g = c // 128
                        ce = min(col0 + pcp, (g + 1) * 128)
                        for q in range(4):
                            nc.scalar.activation(og[g][:, q, c - g * 128:ce - g * 128],
                                                 pu[:, q, c - col0:ce - col0], AF.Tanh,
                                                 bias=pred_bT_t[:, q, :])
                        c = ce

                # level-0 pred: all 16 nodes use v0
                uvT_piece(h0T, _p4(BC), 0, [(v0, 1, ones_f8[:1, :_p4(BC)])], "uv0")

                def logits_group(g):
                    rows = min(128, NCOL - g * 128)
                    lgs = plg.tile([128, V], BF16, tag="lgs", name=f"lgs_{g}")
                    sums = pls.tile([128, NV], F32, tag="sums", name=f"sums_{g}")
                    for n in range(NV):
                        pg = psg.tile([128, 512], F32, space="PSUM", tag="pg")
                        for q2 in range(2):
                            nc.tensor.matmul(pg[:, :VC], og[g][:, 2 * q2:2 * q2 + 2, :],
                                             lwT_t[:, 2 * q2:2 * q2 + 2, n * VC:(n + 1) * VC],
                                             start=(q2 == 0), stop=False,
                                             perf_mode=mybir.MatmulPerfMode.DoubleRow)
                        nc.tensor.matmul(pg[:, :VC], ones_f8[:1, :128],
                                         lb_t[:1, n * VC:(n + 1) * VC], start=False, stop=True)
                        nc.vector.tensor_copy(lgs[:, n * VC:(n + 1) * VC], pg[:, :VC])
                        esc = pls.tile([128, VC], BF16, tag="esc")
                        nc.scalar.activation(esc[:, :], pg[:, :VC], AF.Exp,
                                             accum_out=sums[:, n:n + 1])
                    lse = pls.tile([128, 2], F32, tag="lse", name=f"lse_{g}")
                    nc.vector.tensor_reduce(out=lse[:, 0:1], in_=sums[:, :],
                                            axis=mybir.AxisListType.X, op=OP.add)
                    nc.scalar.activation(lse[:, 1:2], lse[:, 0:1], AF.Ln)
                    for s in range(NST):
                        oc = poc.tile([128, SC], BF16, tag="oc")
                        nc.vector.tensor_scalar(out=oc[:rows, :],
                                                in0=lgs[:rows, s * SC:(s + 1) * SC],
                                                scalar1=lse[:rows, 1:2], scalar2=None,
                                                op0=OP.subtract)
                        nc.gpsimd.dma_start(OUT[g * 128:g * 128 + rows, s * SC:(s + 1) * SC],
                                            oc[:rows, :])

                grp_ready_level = []
                for g in range(NGRP):
                    end = (g + 1) * 128
                    lv = 0
                    for l in range(1, NLV + 1):
                        if 16 + int(OL[l - 1]) < end:
                            lv = l
                    grp_ready_level.append(lv)
                done_groups = set()

                # ---------------- ancestral levels ----------------
                prev_pieces = [(hc2_0, BC)]
                for l in range(1, NLV + 1):
                    sel_t = sel_ts[l - 1]
                    sv_t = selv_ts[l - 1]
                    new_pieces = []
                    for pi, (o_lvl, pc) in enumerate(_chunks(NL[l - 1])):
                        po = int(OL[l - 1]) + o_lvl
                        pcp = min(_p4(pc), 128)
                        xa_t = pxa.tile([128, G], BF16, tag="xat")
                        if l <= LTAIL:
                            nc.sync.dma_start(xa_t[:pc, :], XAl[l - 1][o_lvl:o_lvl + pc, :])
                        else:
                            to = tail_off[l] + o_lvl
                            nc.sync.dma_start(xa_t[:pc, :], XAt[to:to + pc, :])
                        haT = pw2.tile([128, 4, 128], F8, tag="haT")
                        ph = pgh.tile([128, 4, 128], F32, space="PSUM", tag="ghx")
                        for mm in range(4):
                            for kj, (hrp, pck) in enumerate(prev_pieces):
                                nc.tensor.matmul(ph[:, mm, :pcp], hrp[:pck, mm * 128:(mm + 1) * 128],
                                                 sel_t[:pck, kj, o_lvl:o_lvl + pcp],
                                                 start=(kj == 0), stop=(kj == len(prev_pieces) - 1))
                        nc.vector.tensor_copy(haT[:, :, :pcp], ph[:, :, :pcp])
                        cgp = pcg.tile([128, 512], F32, space="PSUM", tag="cgp")
                        for kj, (hrp, pck) in enumerate(prev_pieces):
                            nc.tensor.matmul(cgp[:pc, :], sel_t[:pck, kj, o_lvl:o_lvl + pc],
                                             hrp[:pck, H:2 * H],
                                             start=(kj == 0), stop=(kj == len(prev_pieces) - 1))
                        c_sb = pw2.tile([128, H], BF16, tag="csb")
                        nc.vector.tensor_copy(c_sb[:pc, :], cgp[:pc, :])
                        gs = []
                        for n in range(4):
                            pg = psg.tile([128, 512], F32, space="PSUM", tag="pg")
                            nc.tensor.matmul(pg[:pc, :], ident[:pc, :pc],
                                             xa_t[:pc, n * 512:(n + 1) * 512],
                                             start=True, stop=False)
                            for m2 in range(2):
                                nc.tensor.matmul(pg[:pc, :], haT[:, 2 * m2:2 * m2 + 2, :pc],
                                                 whh_a_t[:, 2 * m2:2 * m2 + 2, n * 512:(n + 1) * 512],
                                                 start=False, stop=(m2 == 1),
                                                 perf_mode=mybir.MatmulPerfMode.DoubleRow)
                            gs.append(pg)
                        gact = pw2.tile([128, G], BF16, tag="gact")
                        hc2 = phc.tile([128, 2 * H], BF16, tag=f"hc2_{(l * 2 + pi) % 4}")
                        tc2 = pw2.tile([128, H], BF16, tag="tc2")
                        elementwise(gs, c_sb, hc2, pc, gact, tc2)
                        new_pieces.append((hc2, pc))
                        hT = pht.tile([128, 4, 128], F8, tag="hT")
                        if pcp > pc:
                            nc.vector.memset(hT[:, :, :], 0.0)
                        pt = pgh.tile([128, 4, 128], BF16, space="PSUM", tag="ghx")
                        for q in range(4):
                            nc.tensor.transpose(pt[:, q, :pc], hc2[:pc, q * 128:(q + 1) * 128],
                                                ident[:pc, :pc])
                        nc.vector.tensor_copy(hT[:, :, :pc], pt[:, :, :pc])
                        vt_jobs = []
                        srcs = sorted(touched[l][pi])
                        for si, src in enumerate(srcs):
                            if src == 4:
                                vt_jobs.append((v0, 1, sv_t[:1, 4, o_lvl:o_lvl + pcp]))
                            else:
                                rows = 128 if src in (0, 2) else (BC * NCH - 128)
                                vt_jobs.append((vsrc[src], rows,
                                                sv_t[:rows, src, o_lvl:o_lvl + pcp]))
                        uvT_piece(hT, pcp, 16 + po, vt_jobs, f"uv_{l}_{pi}")
                    prev_pieces = new_pieces
                    if not SKIP_LOGITS:
                        for g in range(NGRP - 1):
                            if grp_ready_level[g] <= l and g not in done_groups:
                                done_groups.add(g)
                                logits_group(g)

                if not SKIP_LOGITS:
                    for g in range(NGRP):
                        if g not in done_groups:
                            logits_group(g)

    nc.finalize()
    return nc


def _prep(word_idx, father_idx, fc_feats, embed, fc_w, fc_b,
          a_wih, a_whh, a_bih, a_bhh, f_wih, f_whh, f_bih, f_bhh,
          pred_w, pred_b, logit_w, logit_b):
    wi = np.asarray(word_idx).astype(np.int64)
    fa = np.asarray(father_idx).astype(np.int64)
    fc_feats = np.asarray(fc_feats, dtype=np.float32)
    embed = np.asarray(embed, dtype=np.float32)
    L = _levels(fa)
    Lmax = int(L.max())
    NL = []
    for l in range(1, Lmax + 1):
        NL.append(max(int((L[c * BC:(c + 1) * BC] == l).sum()) for c in range(NC_)))
    OL = np.concatenate([[0], np.cumsum(NL)]).astype(int)
    XPAD = int(OL[-1])
    NLP = [_p4(n) for n in NL]

    embT = np.ascontiguousarray(embed.T)              # [E, V]

    def wlay(w, k):   # [D, k*128] -> [128, k, D]T layout as [128, k, D]
        return np.ascontiguousarray(
            np.asarray(w, np.float32).T.reshape(k, 128, -1).transpose(1, 0, 2))

    wih_aT = wlay(a_wih, 4).astype(F8NP)
    wih_fT = wlay(f_wih, 4).astype(F8NP)
    whh_aT = wlay(a_whh, 4).astype(F8NP)
    whh_fT = wlay(f_whh, 4).astype(BF)
    fc_wT = wlay(fc_w, 16).astype(BF)
    paT_ = wlay(pred_w[:, :H], 4).astype(F8NP)
    pfT_ = wlay(pred_w[:, H:], 4).astype(BF)
    lwT_ = wlay(logit_w, 4).astype(F8NP)
    pred_bT_ = np.ascontiguousarray(
        np.asarray(pred_b, np.float32).reshape(4, 128, 1).transpose(1, 0, 2))
    fc_bT_ = np.ascontiguousarray(
        np.asarray(fc_b, np.float32).reshape(4, 128, 1).transpose(1, 0, 2))
    bias_a_ = (np.asarray(a_bih, np.float32) + np.asarray(a_bhh, np.float32)).reshape(1, G).astype(F8NP)
    bias_f_ = (np.asarray(f_bih, np.float32) + np.asarray(f_bhh, np.float32)).reshape(1, G).astype(F8NP)
    logit_b_ = np.asarray(logit_b, np.float32).reshape(1, V).astype(F8NP)

    touched = {l: [set() for _ in _chunks(NL[l - 1])] for l in range(1, Lmax + 1)}
    in_maps = []
    perms = []
    for c in range(NC_):
        gb0 = c * BC
        Lc = L[gb0:gb0 + BC]
        emb_a_ = np.zeros((4, 128, -(-XPAD // 16) * 16), np.float32)
        sels_ = {}
        selv_ = {}
        perm = np.zeros(BC * T, np.int64)
        perm[np.arange(BC) * T] = np.arange(BC)          # i=0 rows
        pos_prev = {(b, 0): b for b in range(BC)}
        for l in range(1, Lmax + 1):
            nodes = [(b, i) for b in range(BC) for i in range(1, T) if Lc[b, i] == l]
            kprev = 1 if l == 1 else len(_chunks(NL[l - 2]))
            sel = np.zeros((kprev, 128, NLP[l - 1]), np.float32)
            sv = np.zeros((5, 128, NLP[l - 1]), np.float32)
            pos_cur = {}
            for j, (b, i) in enumerate(nodes):
                p = int(OL[l - 1]) + j
                pos_cur[(b, i)] = j
                wa = wi[gb0 + b, fa[gb0 + b, i]]
                emb_a_[:, :, p] = embT[:, wa].reshape(4, 128)
                jp = pos_prev[(b, int(fa[gb0 + b, i]))]
                sel[jp // 128, jp % 128, j] = 1.0
                perm[b * T + i] = 16 + p
                pi = j // 128
                if (i - 1) % 3 == 0:
                    sv[4, 0, j] = 1.0
                    touched[l][pi].add(4)
                elif i % 3 == 2:
                    q = b * NCH + (i - 2) // 3
                    src = 0 if q < 128 else 1
                    sv[src, q % 128, j] = 1.0
                    touched[l][pi].add(src)
                else:
                    q = b * NCH + (i - 3) // 3
                    src = 2 if q < 128 else 3
                    sv[src, q % 128, j] = 1.0
                    touched[l][pi].add(src)
            sels_[f"sel_{l}"] = np.ascontiguousarray(sel.transpose(1, 0, 2)).astype(BF)
            selv_[f"selv_{l}"] = np.ascontiguousarray(sv.transpose(1, 0, 2)).astype(F8NP)
            pos_prev = pos_cur
        emb_f_ = np.zeros((4, 128, 512), np.float32)
        for b in range(BC):
            for k in range(NCH):
                p = b * NCH + k
                emb_f_[:, :, p] = embT[:, wi[gb0 + b, 3 * k + 1]].reshape(4, 128)
                emb_f_[:, :, 256 + p] = embT[:, wi[gb0 + b, 3 * k + 2]].reshape(4, 128)
        fcT_ = np.ascontiguousarray(
            fc_feats[gb0:gb0 + BC].T.reshape(16, 128, BC).transpose(1, 0, 2)).astype(BF)

        in_maps.append({
            "emb_a": np.ascontiguousarray(emb_a_.transpose(1, 0, 2)).astype(F8NP),
            "emb_f": np.ascontiguousarray(emb_f_.transpose(1, 0, 2)).astype(F8NP),
            "fcT": fcT_, "fc_wT": fc_wT, "fc_bT": fc_bT_,
            "wih_a": wih_aT, "wih_f": wih_fT, "whh_a": whh_aT, "whh_f": whh_fT,
            "paT": paT_, "pfT": pfT_, "pred_bT": pred_bT_, "lwT": lwT_,
            "logit_b": logit_b_, "bias_a": bias_a_, "bias_f": bias_f_,
            **sels_, **selv_,
        })
        perms.append(perm)
    meta = {"NL": NL, "OL": OL, "XPAD": XPAD, "touched": touched, "perms": perms}
    return in_maps, meta


def kernel(**inputs):
    global LAST_RESULTS, LAST_EXEC_NS
    in_maps, meta = _prep(**inputs)
    nc = _build(meta)
    res = bass_utils.run_bass_kernel_spmd(nc, in_maps, core_ids=list(range(NC_)))
    LAST_RESULTS = res
    LAST_EXEC_NS = res.exec_time_ns
    outs = [res.results[c]["OUT"][meta["perms"][c]].reshape(BC, T, V) for c in range(NC_)]
    return np.concatenate(outs, axis=0).astype(np.float32)


# ---------------------------------------------------------------------------
# Timing helper (not used by grading): paired-timing estimate, see v1 notes.
def _make_runner(nc, in_maps, n_cores=NC_):
    import jax
    from jax.sharding import Mesh, PartitionSpec, NamedSharding
    from concourse import bass2jax

    bass2jax.install_neuronx_cc_hook()
    if nc.dbg_addr is not None:
        in_maps = [{**m, nc.dbg_addr.name: np.zeros((1, 2), np.uint32)} for m in in_maps]
    partition_name = nc.partition_id_tensor.name if nc.partition_id_tensor else None
    in_names, out_names, out_avals, zero_outs = [], [], [], []
    for alloc in nc.m.functions[0].allocations:
        if not isinstance(alloc, mybir.MemoryLocationSet):
            continue
        name = alloc.memorylocations[0].name
        if alloc.kind == "ExternalInput":
            if name != partition_name:
                in_names.append(name)
        elif alloc.kind == "ExternalOutput":
            out_names.append(name)
            shape = tuple(alloc.tensor_shape)
            dtype = mybir.dt.np(alloc.dtype)
            out_avals.append(jax.core.ShapedArray(shape, dtype))
            zero_outs.append(np.zeros(shape, dtype))
    n_params = len(in_names)
    all_in_names = list(in_names) + list(out_names)
    if partition_name is not None:
        all_in_names.append(partition_name)

    def _body(*args):
        operands = list(args)
        if partition_name is not None:
            operands.append(bass2jax.partition_id_tensor())
        outs = bass2jax._bass_exec_p.bind(
            *operands, out_avals=tuple(out_avals), in_names=tuple(all_in_names),
            out_names=tuple(out_names), lowering_input_output_aliases=(),
            sim_require_finite=True, sim_require_nnan=True, nc=nc)
        return tuple(outs)

    devices = jax.devices()[:n_cores]
    mesh = Mesh(np.asarray(devices), ("core",))
    in_specs = (PartitionSpec("core"),) * (n_params + len(out_names))
    out_specs = (PartitionSpec("core"),) * len(out_names)
    sharded = jax.jit(
        jax.shard_map(_body, mesh=mesh, in_specs=in_specs, out_specs=out_specs,
                      check_vma=False), keep_unused=True)
    concat_in = [np.concatenate([np.asarray(in_maps[c][nm]) for c in range(n_cores)], axis=0)
                 for nm in in_names]
    concat_zeros = [np.zeros((n_cores * z.shape[0], *z.shape[1:]), z.dtype) for z in zero_outs]
    sh = NamedSharding(mesh, PartitionSpec("core"))
    dev_args = [jax.device_put(x, sh) for x in concat_in + concat_zeros]
    return sharded, dev_args


def _trivial_nc():
    nc = bacc.Bacc("TRN2", target_bir_lowering=False, debug=True)
    x = nc.dram_tensor("x", [128, 512], F32, kind="ExternalInput")
    y = nc.dram_tensor("y", [128, 512], F32, kind="ExternalOutput")
    with tile.TileContext(nc) as tc:
        with tc.tile_pool(name="sb", bufs=2) as pool:
            t = pool.tile([128, 512], F32)
            nc.sync.dma_start(t[:], x[:])
            t2 = pool.tile([128, 512], F32)
            nc.scalar.mul(t2[:], t[:], 2.0)
            nc.sync.dma_start(y[:], t2[:])
    nc.finalize()
    im = [{"x": np.zeros((128, 512), np.float32)} for _ in range(NC_)]
    return nc, im


def bench_ns(inputs, pairs=40):
    import time
    import jax
    in_maps, meta = _prep(**inputs)
    nc = _build(meta)
    run_k, args_k = _make_runner(nc, in_maps)
    tnc, tim = _trivial_nc()
    run_t, args_t = _make_runner(tnc, tim)
    jax.block_until_ready(run_k(*args_k))
    jax.block_until_ready(run_t(*args_t))
    dk, dt = [], []
    for _ in range(pairs):
        t0 = time.perf_counter()
        jax.block_until_ready(run_t(*args_t))
        t1 = time.perf_counter()
        jax.block_until_ready(run_k(*args_k))
        t2 = time.perf_counter()
        dt.append(t1 - t0)
        dk.append(t2 - t1)
    dk, dt = np.array(dk), np.array(dt)
    est = np.median(dk) - np.median(dt)
    est_min = dk.min() - dt.min()
    return int(est * 1e9), int(est_min * 1e9)


# revision 6
# speedup vs baseline: 1.8468x; 1.3377x over previous
"""Trainium2 Bass kernel for nn_DRNN (tree double-LSTM decoder + logits/log_softmax).

v2 strategy (vs v1):
  - Pure data parallel: batch B=128 sharded 16 rows/core over 8 cores.
  - Everything stays in LEVEL ORDER on device; the host permutes OUT rows
    back to natural (b, t) order after the run. This kills the HC/HF DRAM
    round trips, indirect scatters and the 40 pred-head transposes, and lets
    the logits phase start while late tree levels are still running.
  - bf16 state pipeline (h, c, gates add, selections); fp8 logits weights
    kept fully SBUF-resident so the logits loop runs group-outer with the
    log-softmax store pipelined per group (no un-overlapped store tail).
  - Weights are host-pre-transposed to [128, ...] contiguous layouts (single
    big DMAs, no rearrange descriptor storms).
  - pred head computed transposed per level piece via matmuls:
      uvT[q, col] = P_a @ h_a^T (from own-piece transposes)
                  + P_f @ h_f^T (gathered from fraternal round outputs via
                    host-baked 0/1 selection matmuls) ; tanh(+pred_b) -> outT.
"""

import sys

sys.path.insert(0, "/opt/trn_rl_repo")

import numpy as np
import ml_dtypes

import concourse.bass as bass
import concourse.bacc as bacc
import concourse.tile as tile
from concourse import mybir
from concourse import bass_utils
from concourse.masks import make_identity

F32 = mybir.dt.float32
BF16 = mybir.dt.bfloat16
F8 = mybir.dt.float8e4
AF = mybir.ActivationFunctionType
OP = mybir.AluOpType

B, T, E, H, V, FC = 128, 40, 512, 512, 10000, 2048
NC_, BC = 8, 16          # cores, batch per core
G = 4 * H                # 2048 gate dim
NV = 20                  # logits column chunks (matmul)
VC = V // NV             # 500
NST = 4                  # store chunks per group
SC = V // NST            # 1250
NCH = 13                 # fraternal chains per row

BF = ml_dtypes.bfloat16
F8NP = ml_dtypes.float8_e4m3

LAST_RESULTS = None
LAST_EXEC_NS = None
SKIP_LOGITS = False


def _levels(fa):
    L = np.zeros((B, T), dtype=np.int32)
    rows = np.arange(B)
    for i in range(1, T):
        L[:, i] = 1 + L[rows, fa[:, i]]
    return L


def _chunks(n, step=128):
    out = []
    o = 0
    while o < n:
        out.append((o, min(step, n - o)))
        o += step
    return out


def _p4(n):
    return -(-n // 4) * 4


def _p16(n):
    return -(-n // 16) * 16


def _build(meta):
    NL, OL, XPAD = meta["NL"], meta["OL"], meta["XPAD"]
    touched = meta["touched"]   # {level: [set(src) per piece]}
    NLV = len(NL)
    NLP = [_p4(n) for n in NL]
    KPREV = [1] + [len(_chunks(NL[l])) for l in range(NLV - 1)]
    NCOL = 16 + XPAD
    NGRP = -(-NCOL // 128)
    # fraternal chunks (208 rows each round)
    FCH = _chunks(BC * NCH)

    nc = bacc.Bacc("TRN2", target_bir_lowering=False, debug=True)

    def din(name, shape, dt):
        return nc.dram_tensor(name, list(shape), dt, kind="ExternalInput")

    emb_a = din("emb_a", [128, 4, _p16(XPAD)], F8)
    emb_f = din("emb_f", [128, 4, 512], F8)
    fcT = din("fcT", [128, 16, BC], F8)
    fc_wT = din("fc_wT", [128, 16, H], F8)
    fc_bT = din("fc_bT", [128, 4, 1], F32)
    wih_a = din("wih_a", [128, 4, G], F8)
    wih_f = din("wih_f", [128, 4, G], F8)
    whh_a = din("whh_a", [128, 4, G], F8)
    whh_f = din("whh_f", [128, 4, G], F8)
    paT = din("paT", [128, 4, H], F8)
    pfT = din("pfT", [128, 4, H], F8)
    pred_bT = din("pred_bT", [128, 4, 1], F32)
    lwT = din("lwT", [128, 4, V], F8)
    logit_b = din("logit_b", [1, V], F8)
    bias_a = din("bias_a", [1, G], F8)
    bias_f = din("bias_f", [1, G], F8)
    sels = [din(f"sel_{l+1}", [128, KPREV[l], NLP[l]], BF16) for l in range(NLV)]
    selv = [din(f"selv_{l+1}", [128, 5, NLP[l]], F8) for l in range(NLV)]

    OUT = nc.dram_tensor("OUT", [NGRP * 128, V], F32, kind="ExternalOutput")

    with tile.TileContext(nc) as tc:
        with tc.tile_pool(name="p0", bufs=1) as p0, \
             tc.tile_pool(name="dram", bufs=1, space="DRAM") as pd, \
             tc.tile_pool(name="psg", bufs=4, space="PSUM") as psg, \
             tc.tile_pool(name="pst", bufs=1, space="PSUM") as pst, \
             tc.tile_pool(name="pcg", bufs=1, space="PSUM") as pcg, \
             tc.tile_pool(name="pgh", bufs=1, space="PSUM") as pgh, \
             tc.tile_pool(name="puv", bufs=1, space="PSUM") as puv, \
             tc.tile_pool(name="pxa", bufs=3) as pxa, \
             tc.tile_pool(name="pfr", bufs=1) as pfr, \
             tc.tile_pool(name="pw2", bufs=2) as pw2, \
             tc.tile_pool(name="phc", bufs=1) as phc, \
             tc.tile_pool(name="pht", bufs=2) as pht, \
             tc.tile_pool(name="pls", bufs=2) as pls:

            XF = pd.tile([512, G], BF16)
            LTAIL = 5                      # levels > LTAIL share one projection tensor
            XAl = [pd.tile([max(NL[l], 4), G], BF16, name=f"XA_{l+1}")
                   for l in range(min(LTAIL, NLV))]
            tail_rows = sum(NL[LTAIL:]) if NLV > LTAIL else 0
            XAt = None
            if tail_rows:
                XAt = pd.tile([max(tail_rows, 4), G], BF16, name="XA_tail")
            tail_off = {}
            off = 0
            for l in range(LTAIL + 1, NLV + 1):
                tail_off[l] = off
                off += NL[l - 1]

            ident = p0.tile([128, 128], BF16)
            make_identity(nc, ident[:])
            ones_bf = p0.tile([1, 128], BF16)
            nc.vector.memset(ones_bf[:], 1.0)
            ones_f8 = p0.tile([1, 128], F8)
            nc.vector.memset(ones_f8[:], 1.0)

            paT_t = p0.tile([128, 4, H], F8)
            pfT_t = p0.tile([128, 4, H], F8)
            nc.sync.dma_start(paT_t[:], paT[:])
            nc.sync.dma_start(pfT_t[:], pfT[:])
            pred_bT_t = p0.tile([128, 4, 1], F32)
            nc.sync.dma_start(pred_bT_t[:], pred_bT[:])
            whh_a_t = p0.tile([128, 4, G], F8)
            whh_f_t = p0.tile([128, 4, G], F8)
            sel_ts = [p0.tile([128, KPREV[l], NLP[l]], BF16, name=f"sel_t{l}")
                      for l in range(NLV)]
            selv_ts = [p0.tile([128, 5, NLP[l]], F8, name=f"selv_t{l}")
                       for l in range(NLV)]

            og = []
            for g in range(NGRP):
                t = p0.tile([128, 4, 128], F8, name=f"og{g}")
                nc.vector.memset(t[:], 0.0)
                og.append(t)

            xa0T = p0.tile([128, 4, BC], F8)
            cf0_bf = p0.tile([128, H], BF16)
            hf0 = p0.tile([1, H], BF16)
            hf0T = p0.tile([128, 4, 1], F8)
            w0f = p0.tile([1, G], BF16)
            v0 = p0.tile([1, H], F8)
            vsrc = [p0.tile([128, H], F8, name=f"vsrc{j}") for j in range(4)]
            hc2_0 = p0.tile([BC, 2 * H], BF16)
            h0T = p0.tile([128, 4, _p4(BC)], F8)
            hs1 = [p0.tile([128, 2 * H], BF16, name=f"hs1_{j}") for j in range(2)]
            hs1T = [p0.tile([128, 4, 128], F8, name=f"hs1T_{j}") for j in range(2)]

            def elementwise(gs, c_in, hc2, pc, gact, tc2):
                """gates gs = 4 psum tiles [pc, 512] (i f g o), c_in [pc, H] -> hc2 bf16.
                sigmoid(x) = 0.5*tanh(x/2) + 0.5 so only the tanh act table is used;
                the affine is fused into one tensor_scalar per gate on DVE."""
                for k, sc in ((0, 0.5), (1, 0.5), (2, 1.0), (3, 0.5)):
                    nc.scalar.activation(gact[:pc, k * H:(k + 1) * H], gs[k][:pc, :],
                                         AF.Tanh, scale=sc)
                nc.vector.tensor_scalar(out=gact[:pc, 0:2 * H], in0=gact[:pc, 0:2 * H],
                                        scalar1=1.0, scalar2=0.5, op0=OP.add, op1=OP.mult)
                nc.vector.tensor_scalar(out=gact[:pc, 3 * H:4 * H], in0=gact[:pc, 3 * H:4 * H],
                                        scalar1=1.0, scalar2=0.5, op0=OP.add, op1=OP.mult)
                if c_in is not None:
                    nc.vector.tensor_tensor(out=hc2[:pc, H:2 * H], in0=gact[:pc, H:2 * H],
                                            in1=c_in[:pc, :], op=OP.mult)
                    nc.vector.tensor_tensor(out=tc2[:pc, :], in0=gact[:pc, 0:H],
                                            in1=gact[:pc, 2 * H:3 * H], op=OP.mult)
                    nc.vector.tensor_tensor(out=hc2[:pc, H:2 * H], in0=hc2[:pc, H:2 * H],
                                            in1=tc2[:pc, :], op=OP.add)
                else:
                    nc.vector.tensor_tensor(out=hc2[:pc, H:2 * H], in0=gact[:pc, 0:H],
                                            in1=gact[:pc, 2 * H:3 * H], op=OP.mult)
                nc.scalar.activation(tc2[:pc, :], hc2[:pc, H:2 * H], AF.Tanh)
                nc.vector.tensor_tensor(out=hc2[:pc, 0:H], in0=gact[:pc, 3 * H:4 * H],
                                        in1=tc2[:pc, :], op=OP.mult)

            # ---- early scope: fc, hf0 const, projections, L0 (space reused later) ----
            with tc.tile_pool(name="pe1", bufs=1) as pe1, \
                 tc.tile_pool(name="pw1", bufs=2) as pw1, \
                 tc.tile_pool(name="pxc", bufs=8) as pxc:
                fcT_t = pe1.tile([128, 16, BC], F8)
                fc_wT_t = pe1.tile([128, 16, H], F8)
                fc_bT_t = pe1.tile([128, 4, 1], F32)
                nc.sync.dma_start(fcT_t[:], fcT[:])
                nc.sync.dma_start(fc_wT_t[:], fc_wT[:])
                nc.sync.dma_start(fc_bT_t[:], fc_bT[:])
                bias_a_t = pe1.tile([1, G], F8)
                nc.sync.dma_start(bias_a_t[:], bias_a[:])
                bias_f_t = pe1.tile([1, G], F8)
                nc.sync.dma_start(bias_f_t[:], bias_f[:])
                emb_a_t = pe1.tile([128, 4, _p16(XPAD)], F8)
                emb_f_t = pe1.tile([128, 4, 512], F8)
                wih_a_t = pe1.tile([128, 4, G], F8)
                wih_f_t = pe1.tile([128, 4, G], F8)
                nc.sync.dma_start(emb_a_t[:], emb_a[:])
                nc.sync.dma_start(wih_a_t[:], wih_a[:])
                nc.sync.dma_start(emb_f_t[:], emb_f[:])
                nc.sync.dma_start(wih_f_t[:], wih_f[:])
                nc.sync.dma_start(whh_f_t[:], whh_f[:])
                nc.sync.dma_start(whh_a_t[:], whh_a[:])
                for l in range(NLV):
                    nc.sync.dma_start(sel_ts[l][:], sels[l][:])
                    nc.sync.dma_start(selv_ts[l][:], selv[l][:])

                # fc path
                for mm in range(4):
                    pp = pst.tile([128, BC], F32, space="PSUM", tag="ptr")
                    for q in range(16):
                        nc.tensor.matmul(pp[:, :], fc_wT_t[:, q, mm * 128:(mm + 1) * 128],
                                         fcT_t[:, q, :], start=(q == 0), stop=(q == 15))
                    nc.scalar.activation(xa0T[:, mm, :], pp[:, :], AF.Identity,
                                         bias=fc_bT_t[:, mm, :])

                # hf0 = LSTM(0 input, 0 state) from biases only
                gactc = pe1.tile([128, G], BF16)
                for n in range(4):
                    pg = psg.tile([128, 512], F32, space="PSUM", tag="pg")
                    nc.tensor.matmul(pg[:, :], ones_f8[:1, :128],
                                     bias_f_t[:1, n * 512:(n + 1) * 512], start=True, stop=True)
                    sc_ = 1.0 if n == 2 else 0.5
                    nc.scalar.activation(gactc[:, n * 512:(n + 1) * 512], pg[:, :],
                                         AF.Tanh, scale=sc_)
                for k in (0, 1, 3):
                    nc.vector.tensor_scalar(out=gactc[:, k * 512:(k + 1) * 512],
                                            in0=gactc[:, k * 512:(k + 1) * 512],
                                            scalar1=1.0, scalar2=0.5,
                                            op0=OP.add, op1=OP.mult)
                nc.vector.tensor_tensor(out=cf0_bf[:, :], in0=gactc[:, 0:H],
                                        in1=gactc[:, 2 * H:3 * H], op=OP.mult)
                tcf0 = pe1.tile([128, H], BF16)
                nc.scalar.activation(tcf0[:, :], cf0_bf[:, :], AF.Tanh)
                nc.vector.tensor_tensor(out=hf0[:1, :], in0=gactc[:1, 3 * H:4 * H],
                                        in1=tcf0[:1, :], op=OP.mult)
                for q in range(4):
                    pt = pst.tile([128, 128], BF16, space="PSUM", tag="ptr")
                    nc.tensor.transpose(pt[:, :1], hf0[0:1, q * 128:(q + 1) * 128], ident[:1, :1])
                    nc.vector.tensor_copy(hf0T[:, q, :], pt[:, :1])
                for n in range(4):
                    pg = psg.tile([128, 512], F32, space="PSUM", tag="pg")
                    for q in range(4):
                        nc.tensor.matmul(pg[:1, :], hf0T[:, q, :],
                                         whh_f_t[:, q, n * 512:(n + 1) * 512],
                                         start=(q == 0), stop=(q == 3))
                    nc.vector.tensor_copy(w0f[:1, n * 512:(n + 1) * 512], pg[:1, :])
                pg = psg.tile([128, 512], F32, space="PSUM", tag="pg")
                for q in range(4):
                    nc.tensor.matmul(pg[:1, :], hf0T[:, q, :], pfT_t[:, q, :],
                                     start=(q == 0), stop=(q == 3))
                nc.vector.tensor_copy(v0[:1, :], pg[:1, :])

                def proj(src_t, w, bias_row, dst, base, rows):
                    for (ro, rc) in rows:
                        for n in range(4):
                            pg = psg.tile([128, 512], F32, space="PSUM", tag="pg")
                            for q2 in range(2):
                                nc.tensor.matmul(pg[:rc, :],
                                                 src_t[:, 2 * q2:2 * q2 + 2, ro:ro + rc],
                                                 w[:, 2 * q2:2 * q2 + 2, n * 512:(n + 1) * 512],
                                                 start=(q2 == 0), stop=False,
                                                 perf_mode=mybir.MatmulPerfMode.DoubleRow)
                            nc.tensor.matmul(pg[:rc, :], ones_f8[:1, :rc],
                                             bias_row[:1, n * 512:(n + 1) * 512],
                                             start=False, stop=True)
                            xc = pxc.tile([128, 512], BF16, tag="xc")
                            if n % 2 == 0:
                                nc.vector.tensor_copy(xc[:rc, :], pg[:rc, :])
                            else:
                                nc.scalar.copy(xc[:rc, :], pg[:rc, :])
                            nc.sync.dma_start(dst[ro - base:ro - base + rc,
                                                  n * 512:(n + 1) * 512], xc[:rc, :])

                def proj_level(l):
                    if l > LTAIL:
                        return
                    proj(emb_a_t, wih_a_t, bias_a_t, XAl[l - 1], int(OL[l - 1]),
                         [(int(OL[l - 1]) + o, c) for (o, c) in _chunks(NL[l - 1])])

                # XA_1 (level 1) first, then L0 gates, XF (fraternal), then the rest
                proj_level(1)

                g0s = []
                for n in range(4):
                    pg = psg.tile([128, 512], F32, space="PSUM", tag="pg")
                    for q2 in range(2):
                        nc.tensor.matmul(pg[:BC, :], xa0T[:, 2 * q2:2 * q2 + 2, :],
                                         wih_a_t[:, 2 * q2:2 * q2 + 2, n * 512:(n + 1) * 512],
                                         start=(q2 == 0), stop=False,
                                         perf_mode=mybir.MatmulPerfMode.DoubleRow)
                    nc.tensor.matmul(pg[:BC, :], ones_f8[:1, :BC],
                                     bias_a_t[:1, n * 512:(n + 1) * 512],
                                     start=False, stop=True)
                    g0s.append(pg)
                gact0 = pw1.tile([BC, G], BF16, tag="gact0")
                tc20 = pw1.tile([BC, H], BF16, tag="tc20")
                elementwise(g0s, None, hc2_0, BC, gact0, tc20)
                nc.vector.memset(h0T[:], 0.0)
                for q in range(4):
                    pt = pst.tile([128, 128], BF16, space="PSUM", tag="ptr")
                    nc.tensor.transpose(pt[:, :BC], hc2_0[:BC, q * 128:(q + 1) * 128],
                                        ident[:BC, :BC])
                    nc.vector.tensor_copy(h0T[:, q, :BC], pt[:, :BC])

                proj(emb_f_t, wih_f_t, bias_f_t, XF, 0, _chunks(512))
                for l in range(2, NLV + 1):
                    proj_level(l)
                if XAt is not None:
                    base = int(OL[LTAIL])
                    proj(emb_a_t, wih_a_t, bias_a_t, XAt, base,
                         [(base + o, c) for (o, c) in _chunks(sum(NL[LTAIL:]))])

            # ---- late pool: logits weights + buffers (reuses early-scope space) ----
            with tc.tile_pool(name="plt", bufs=1) as plt, \
                 tc.tile_pool(name="plg", bufs=2) as plg, \
                 tc.tile_pool(name="poc", bufs=2) as poc:
                lwT_t = plt.tile([128, 4, V], F8)
                nc.gpsimd.dma_start(lwT_t[:], lwT[:])
                lb_t = plt.tile([1, V], F8)
                nc.gpsimd.dma_start(lb_t[:], logit_b[:])

                # ---- fraternal rounds ----
                def frat_s1(j, o, c):
                    xf_t = pfr.tile([128, G], BF16, tag="fxat", name=f"xf1_{j}")
                    nc.sync.dma_start(xf_t[:c, :], XF[o:o + c, :])
                    gs = []
                    for n in range(4):
                        pg = psg.tile([128, 512], F32, space="PSUM", tag="pg")
                        nc.tensor.matmul(pg[:c, :], ones_bf[:1, :c],
                                         w0f[:1, n * 512:(n + 1) * 512], start=True, stop=False)
                        nc.tensor.matmul(pg[:c, :], ident[:c, :c],
                                         xf_t[:c, n * 512:(n + 1) * 512], start=False, stop=True)
                        gs.append(pg)
                    gact = pfr.tile([128, G], BF16, tag="fgact")
                    tc2 = pfr.tile([128, H], BF16, tag="ftc2")
                    elementwise(gs, cf0_bf, hs1[j], c, gact, tc2)
                    for q in range(4):
                        pt = pst.tile([128, 128], BF16, space="PSUM", tag="ptr")
                        nc.tensor.transpose(pt[:, :c], hs1[j][:c, q * 128:(q + 1) * 128],
                                            ident[:c, :c])
                        nc.vector.tensor_copy(hs1T[j][:, q, :c], pt[:, :c])
                    pg = psg.tile([128, 512], F32, space="PSUM", tag="pg")
                    for q in range(4):
                        nc.tensor.matmul(pg[:c, :], hs1T[j][:, q, :c], pfT_t[:, q, :],
                                         start=(q == 0), stop=(q == 3))
                    nc.vector.tensor_copy(vsrc[j][:c, :], pg[:c, :])

                def frat_s2(j, o, c):
                    xf_t = pfr.tile([128, G], BF16, tag="fxat", name=f"xf2_{j}")
                    nc.sync.dma_start(xf_t[:c, :], XF[256 + o:256 + o + c, :])
                    gs = []
                    for n in range(4):
                        pg = psg.tile([128, 512], F32, space="PSUM", tag="pg")
                        nc.tensor.matmul(pg[:c, :], ident[:c, :c],
                                         xf_t[:c, n * 512:(n + 1) * 512], start=True, stop=False)
                        for m2 in range(2):
                            nc.tensor.matmul(pg[:c, :], hs1T[j][:, 2 * m2:2 * m2 + 2, :c],
                                             whh_f_t[:, 2 * m2:2 * m2 + 2, n * 512:(n + 1) * 512],
                                             start=False, stop=(m2 == 1),
                                             perf_mode=mybir.MatmulPerfMode.DoubleRow)
                        gs.append(pg)
                    gact = pfr.tile([128, G], BF16, tag="fgact")
                    hc2 = pfr.tile([128, 2 * H], BF16, tag="hs2", name=f"hs2_{j}")
                    tc2 = pfr.tile([128, H], BF16, tag="ftc2")
                    elementwise(gs, hs1[j][:, H:2 * H], hc2, c, gact, tc2)
                    hT = pfr.tile([128, 4, 128], F8, tag="fhT", name=f"hs2T_{j}")
                    for q in range(4):
                        pt = pst.tile([128, 128], BF16, space="PSUM", tag="ptr")
                        nc.tensor.transpose(pt[:, :c], hc2[:c, q * 128:(q + 1) * 128],
                                            ident[:c, :c])
                        nc.vector.tensor_copy(hT[:, q, :c], pt[:, :c])
                    pg = psg.tile([128, 512], F32, space="PSUM", tag="pg")
                    for q in range(4):
                        nc.tensor.matmul(pg[:c, :], hT[:, q, :c], pfT_t[:, q, :],
                                         start=(q == 0), stop=(q == 3))
                    nc.vector.tensor_copy(vsrc[2 + j][:c, :], pg[:c, :])

                for j, (o, c) in enumerate(FCH):
                    frat_s1(j, o, c)
                for j, (o, c) in enumerate(FCH):
                    frat_s2(j, o, c)

                def uvT_piece(hT, pcp, col0, vT_jobs, name):
                    """pred-head transposed for one piece: cols [col0, col0+pcp) of outT"""
                    pu = puv.tile([128, 4, 128], F32, space="PSUM", tag="puv")
                    for q in range(4):
                        for k2 in range(2):
                            nc.tensor.matmul(pu[:, q, :pcp],
                                             paT_t[:, 2 * k2:2 * k2 + 2, q * 128:(q + 1) * 128],
                                             hT[:, 2 * k2:2 * k2 + 2, :pcp],
                                             start=(k2 == 0), stop=False,
                                             perf_mode=mybir.MatmulPerfMode.DoubleRow)
                        for ji, (lhs, rows, rhs) in enumerate(vT_jobs):
                            nc.tensor.matmul(pu[:, q, :pcp],
                                             lhs[:rows, q * 128:(q + 1) * 128],
                                             rhs[:, :pcp],
                                             start=False, stop=(ji == len(vT_jobs) - 1))
                    c = col0
                    while c < col0 + pcp:
                        g = c // 128
                        ce = min(col0 + pcp, (g + 1) * 128)
                        for q in range(4):
                            nc.scalar.activation(og[g][:, q, c - g * 128:ce - g * 128],
                                                 pu[:, q, c - col0:ce - col0], AF.Tanh,
                                                 bias=pred_bT_t[:, q, :])
                        c = ce

                # level-0 pred: all 16 nodes use v0
                uvT_piece(h0T, _p4(BC), 0, [(v0, 1, ones_f8[:1, :_p4(BC)])], "uv0")

                def logits_group(g):
                    rows = min(128, NCOL - g * 128)
                    lgs = plg.tile([128, V], BF16, tag="lgs", name=f"lgs_{g}")
                    sums = pls.tile([128, NV], F32, tag="sums", name=f"sums_{g}")
                    for n in range(NV):
                        pg = psg.tile([128, 512], F32, space="PSUM", tag="pg")
                        for q2 in range(2):
                            nc.tensor.matmul(pg[:, :VC], og[g][:, 2 * q2:2 * q2 + 2, :],
                                             lwT_t[:, 2 * q2:2 * q2 + 2, n * VC:(n + 1) * VC],
                                             start=(q2 == 0), stop=False,
                                             perf_mode=mybir.MatmulPerfMode.DoubleRow)
                        nc.tensor.matmul(pg[:, :VC], ones_f8[:1, :128],
                                         lb_t[:1, n * VC:(n + 1) * VC], start=False, stop=True)
                        nc.vector.tensor_copy(lgs[:, n * VC:(n + 1) * VC], pg[:, :VC])
                        esc = pls.tile([128, VC], BF16, tag="esc")
                        nc.scalar.activation(esc[:, :], pg[:, :VC], AF.Exp,
                                             accum_out=sums[:, n:n + 1])
                    lse = pls.tile([128, 2], F32, tag="lse", name=f"lse_{g}")
                    nc.vector.tensor_reduce(out=lse[:, 0:1], in_=sums[:, :],
                                            axis=mybir.AxisListType.X, op=OP.add)
                    nc.scalar.activation(lse[:, 1:2], lse[:, 0:1], AF.Ln)
                    for s in range(NST):
                        oc = poc.tile([128, SC], BF16, tag="oc")
                        nc.vector.tensor_scalar(out=oc[:rows, :],
                                                in0=lgs[:rows, s * SC:(s + 1) * SC],
                                                scalar1=lse[:rows, 1:2], scalar2=None,
                                                op0=OP.subtract)
                        nc.gpsimd.dma_start(OUT[g * 128:g * 128 + rows, s * SC:(s + 1) * SC],
                                            oc[:rows, :])

                grp_ready_level = []
                for g in range(NGRP):
                    end = (g + 1) * 128
                    lv = 0
                    for l in range(1, NLV + 1):
                        if 16 + int(OL[l - 1]) < end:
                            lv = l
                    grp_ready_level.append(lv)
                done_groups = set()

                # ---------------- ancestral levels ----------------
                prev_pieces = [(hc2_0, BC)]
                for l in range(1, NLV + 1):
                    sel_t = sel_ts[l - 1]
                    sv_t = selv_ts[l - 1]
                    new_pieces = []
                    for pi, (o_lvl, pc) in enumerate(_chunks(NL[l - 1])):
                        po = int(OL[l - 1]) + o_lvl
                        pcp = min(_p4(pc), 128)
                        xa_t = pxa.tile([128, G], BF16, tag="xat")
                        if l <= LTAIL:
                            nc.sync.dma_start(xa_t[:pc, :], XAl[l - 1][o_lvl:o_lvl + pc, :])
                        else:
                            to = tail_off[l] + o_lvl
                            nc.sync.dma_start(xa_t[:pc, :], XAt[to:to + pc, :])
                        haT = pw2.tile([128, 4, 128], F8, tag="haT")
                        ph = pgh.tile([128, 4, 128], F32, space="PSUM", tag="ghx")
                        for mm in range(4):
                            for kj, (hrp, pck) in enumerate(prev_pieces):
                                nc.tensor.matmul(ph[:, mm, :pcp], hrp[:pck, mm * 128:(mm + 1) * 128],
                                                 sel_t[:pck, kj, o_lvl:o_lvl + pcp],
                                                 start=(kj == 0), stop=(kj == len(prev_pieces) - 1))
                        nc.vector.tensor_copy(haT[:, :, :pcp], ph[:, :, :pcp])
                        cgp = pcg.tile([128, 512], F32, space="PSUM", tag="cgp")
                        for kj, (hrp, pck) in enumerate(prev_pieces):
                            nc.tensor.matmul(cgp[:pc, :], sel_t[:pck, kj, o_lvl:o_lvl + pc],
                                             hrp[:pck, H:2 * H],
                                             start=(kj == 0), stop=(kj == len(prev_pieces) - 1))
                        c_sb = pw2.tile([128, H], BF16, tag="csb")
                        nc.vector.tensor_copy(c_sb[:pc, :], cgp[:pc, :])
                        gs = []
                        for n in range(4):
                            pg = psg.tile([128, 512], F32, space="PSUM", tag="pg")
                            nc.tensor.matmul(pg[:pc, :], ident[:pc, :pc],
                                             xa_t[:pc, n * 512:(n + 1) * 512],
                                             start=True, stop=False)
                            for m2 in range(2):
                                nc.tensor.matmul(pg[:pc, :], haT[:, 2 * m2:2 * m2 + 2, :pc],
                                                 whh_a_t[:, 2 * m2:2 * m2 + 2, n * 512:(n + 1) * 512],
                                                 start=False, stop=(m2 == 1),
                                                 perf_mode=mybir.MatmulPerfMode.DoubleRow)
                            gs.append(pg)
                        gact = pw2.tile([128, G], BF16, tag="gact")
                        hc2 = phc.tile([128, 2 * H], BF16, tag=f"hc2_{(l * 2 + pi) % 4}")
                        tc2 = pw2.tile([128, H], BF16, tag="tc2")
                        elementwise(gs, c_sb, hc2, pc, gact, tc2)
                        new_pieces.append((hc2, pc))
                        hT = pht.tile([128, 4, 128], F8, tag="hT")
                        if pcp > pc:
                            nc.vector.memset(hT[:, :, :], 0.0)
                        pt = pgh.tile([128, 4, 128], BF16, space="PSUM", tag="ghx")
                        for q in range(4):
                            nc.tensor.transpose(pt[:, q, :pc], hc2[:pc, q * 128:(q + 1) * 128],
                                                ident[:pc, :pc])
                        nc.vector.tensor_copy(hT[:, :, :pc], pt[:, :, :pc])
                        vt_jobs = []
                        srcs = sorted(touched[l][pi])
                        for si, src in enumerate(srcs):
                            if src == 4:
                                vt_jobs.append((v0, 1, sv_t[:1, 4, o_lvl:o_lvl + pcp]))
                            else:
                                rows = 128 if src in (0, 2) else (BC * NCH - 128)
                                vt_jobs.append((vsrc[src], rows,
                                                sv_t[:rows, src, o_lvl:o_lvl + pcp]))
                        uvT_piece(hT, pcp, 16 + po, vt_jobs, f"uv_{l}_{pi}")
                    prev_pieces = new_pieces
                    if not SKIP_LOGITS:
                        for g in range(NGRP - 1):
                            if grp_ready_level[g] <= l and g not in done_groups:
                                done_groups.add(g)
                                logits_group(g)

                if not SKIP_LOGITS:
                    for g in range(NGRP):
                        if g not in done_groups:
                            logits_group(g)

    nc.finalize()
    return nc


def _prep(word_idx, father_idx, fc_feats, embed, fc_w, fc_b,
          a_wih, a_whh, a_bih, a_bhh, f_wih, f_whh, f_bih, f_bhh,
          pred_w, pred_b, logit_w, logit_b):
    wi = np.asarray(word_idx).astype(np.int64)
    fa = np.asarray(father_idx).astype(np.int64)
    fc_feats = np.asarray(fc_feats, dtype=np.float32)
    embed = np.asarray(embed, dtype=np.float32)
    L = _levels(fa)
    Lmax = int(L.max())
    NL = []
    for l in range(1, Lmax + 1):
        NL.append(max(int((L[c * BC:(c + 1) * BC] == l).sum()) for c in range(NC_)))
    OL = np.concatenate([[0], np.cumsum(NL)]).astype(int)
    XPAD = int(OL[-1])
    NLP = [_p4(n) for n in NL]

    embT = np.ascontiguousarray(embed.T)              # [E, V]

    def wlay(w, k):   # [D, k*128] -> [128, k, D]T layout as [128, k, D]
        return np.ascontiguousarray(
            np.asarray(w, np.float32).T.reshape(k, 128, -1).transpose(1, 0, 2))

    wih_aT = wlay(a_wih, 4).astype(F8NP)
    wih_fT = wlay(f_wih, 4).astype(F8NP)
    whh_aT = wlay(a_whh, 4).astype(F8NP)
    whh_fT = wlay(f_whh, 4).astype(F8NP)
    fc_wT = wlay(fc_w, 16).astype(F8NP)
    paT_ = wlay(pred_w[:, :H], 4).astype(F8NP)
    pfT_ = wlay(pred_w[:, H:], 4).astype(F8NP)
    lwT_ = wlay(logit_w, 4).astype(F8NP)
    pred_bT_ = np.ascontiguousarray(
        np.asarray(pred_b, np.float32).reshape(4, 128, 1).transpose(1, 0, 2))
    fc_bT_ = np.ascontiguousarray(
        np.asarray(fc_b, np.float32).reshape(4, 128, 1).transpose(1, 0, 2))
    bias_a_ = (np.asarray(a_bih, np.float32) + np.asarray(a_bhh, np.float32)).reshape(1, G).astype(F8NP)
    bias_f_ = (np.asarray(f_bih, np.float32) + np.asarray(f_bhh, np.float32)).reshape(1, G).astype(F8NP)
    logit_b_ = np.asarray(logit_b, np.float32).reshape(1, V).astype(F8NP)

    touched = {l: [set() for _ in _chunks(NL[l - 1])] for l in range(1, Lmax + 1)}
    in_maps = []
    perms = []
    for c in range(NC_):
        gb0 = c * BC
        Lc = L[gb0:gb0 + BC]
        emb_a_ = np.zeros((4, 128, -(-XPAD // 16) * 16), np.float32)
        sels_ = {}
        selv_ = {}
        perm = np.zeros(BC * T, np.int64)
        perm[np.arange(BC) * T] = np.arange(BC)          # i=0 rows
        pos_prev = {(b, 0): b for b in range(BC)}
        for l in range(1, Lmax + 1):
            nodes = [(b, i) for b in range(BC) for i in range(1, T) if Lc[b, i] == l]
            kprev = 1 if l == 1 else len(_chunks(NL[l - 2]))
            sel = np.zeros((kprev, 128, NLP[l - 1]), np.float32)
            sv = np.zeros((5, 128, NLP[l - 1]), np.float32)
            pos_cur = {}
            for j, (b, i) in enumerate(nodes):
                p = int(OL[l - 1]) + j
                pos_cur[(b, i)] = j
                wa = wi[gb0 + b, fa[gb0 + b, i]]
                emb_a_[:, :, p] = embT[:, wa].reshape(4, 128)
                jp = pos_prev[(b, int(fa[gb0 + b, i]))]
                sel[jp // 128, jp % 128, j] = 1.0
                perm[b * T + i] = 16 + p
                pi = j // 128
                if (i - 1) % 3 == 0:
                    sv[4, 0, j] = 1.0
                    touched[l][pi].add(4)
                elif i % 3 == 2:
                    q = b * NCH + (i - 2) // 3
                    src = 0 if q < 128 else 1
                    sv[src, q % 128, j] = 1.0
                    touched[l][pi].add(src)
                else:
                    q = b * NCH + (i - 3) // 3
                    src = 2 if q < 128 else 3
                    sv[src, q % 128, j] = 1.0
                    touched[l][pi].add(src)
            sels_[f"sel_{l}"] = np.ascontiguousarray(sel.transpose(1, 0, 2)).astype(BF)
            selv_[f"selv_{l}"] = np.ascontiguousarray(sv.transpose(1, 0, 2)).astype(F8NP)
            pos_prev = pos_cur
        emb_f_ = np.zeros((4, 128, 512), np.float32)
        for b in range(BC):
            for k in range(NCH):
                p = b * NCH + k
                emb_f_[:, :, p] = embT[:, wi[gb0 + b, 3 * k + 1]].reshape(4, 128)
                emb_f_[:, :, 256 + p] = embT[:, wi[gb0 + b, 3 * k + 2]].reshape(4, 128)
        fcT_ = np.ascontiguousarray(
            fc_feats[gb0:gb0 + BC].T.reshape(16, 128, BC).transpose(1, 0, 2)).astype(F8NP)

        in_maps.append({
            "emb_a": np.ascontiguousarray(emb_a_.transpose(1, 0, 2)).astype(F8NP),
            "emb_f": np.ascontiguousarray(emb_f_.transpose(1, 0, 2)).astype(F8NP),
            "fcT": fcT_, "fc_wT": fc_wT, "fc_bT": fc_bT_,
            "wih_a": wih_aT, "wih_f": wih_fT, "whh_a": whh_aT, "whh_f": whh_fT,
            "paT": paT_, "pfT": pfT_, "pred_bT": pred_bT_, "lwT": lwT_,
            "logit_b": logit_b_, "bias_a": bias_a_, "bias_f": bias_f_,
            **sels_, **selv_,
        })
        perms.append(perm)
    meta = {"NL": NL, "OL": OL, "XPAD": XPAD, "touched": touched, "perms": perms}
    return in_maps, meta


def kernel(**inputs):
    global LAST_RESULTS, LAST_EXEC_NS
    in_maps, meta = _prep(**inputs)
    nc = _build(meta)
    res = bass_utils.run_bass_kernel_spmd(nc, in_maps, core_ids=list(range(NC_)))
    LAST_RESULTS = res
    LAST_EXEC_NS = res.exec_time_ns
    outs = [res.results[c]["OUT"][meta["perms"][c]].reshape(BC, T, V) for c in range(NC_)]
    return np.concatenate(outs, axis=0).astype(np.float32)


# ---------------------------------------------------------------------------
# Timing helper (not used by grading): paired-timing estimate, see v1 notes.
def _make_runner(nc, in_maps, n_cores=NC_):
    import jax
    from jax.sharding import Mesh, PartitionSpec, NamedSharding
    from concourse import bass2jax

    bass2jax.install_neuronx_cc_hook()
    if nc.dbg_addr is not None:
        in_maps = [{**m, nc.dbg_addr.name: np.zeros((1, 2), np.uint32)} for m in in_maps]
    partition_name = nc.partition_id_tensor.name if nc.partition_id_tensor else None
    in_names, out_names, out_avals, zero_outs = [], [], [], []
    for alloc in nc.m.functions[0].allocations:
        if not isinstance(alloc, mybir.MemoryLocationSet):
            continue
        name = alloc.memorylocations[0].name
        if alloc.kind == "ExternalInput":
            if name != partition_name:
                in_names.append(name)
        elif alloc.kind == "ExternalOutput":
            out_names.append(name)
            shape = tuple(alloc.tensor_shape)
            dtype = mybir.dt.np(alloc.dtype)
            out_avals.append(jax.core.ShapedArray(shape, dtype))
            zero_outs.append(np.zeros(shape, dtype))
    n_params = len(in_names)
    all_in_names = list(in_names) + list(out_names)
    if partition_name is not None:
        all_in_names.append(partition_name)

    def _body(*args):
        operands = list(args)
        if partition_name is not None:
            operands.append(bass2jax.partition_id_tensor())
        outs = bass2jax._bass_exec_p.bind(
            *operands, out_avals=tuple(out_avals), in_names=tuple(all_in_names),
            out_names=tuple(out_names), lowering_input_output_aliases=(),
            sim_require_finite=True, sim_require_nnan=True, nc=nc)
        return tuple(outs)

    devices = jax.devices()[:n_cores]
    mesh = Mesh(np.asarray(devices), ("core",))
    in_specs = (PartitionSpec("core"),) * (n_params + len(out_names))
    out_specs = (PartitionSpec("core"),) * len(out_names)
    sharded = jax.jit(
        jax.shard_map(_body, mesh=mesh, in_specs=in_specs, out_specs=out_specs,
                      check_vma=False), keep_unused=True)
    concat_in = [np.concatenate([np.asarray(in_maps[c][nm]) for c in range(n_cores)], axis=0)
                 for nm in in_names]
    concat_zeros = [np.zeros((n_cores * z.shape[0], *z.shape[1:]), z.dtype) for z in zero_outs]
    sh = NamedSharding(mesh, PartitionSpec("core"))
    dev_args = [jax.device_put(x, sh) for x in concat_in + concat_zeros]
    return sharded, dev_args


def _trivial_nc():
    nc = bacc.Bacc("TRN2", target_bir_lowering=False, debug=True)
    x = nc.dram_tensor("x", [128, 512], F32, kind="ExternalInput")
    y = nc.dram_tensor("y", [128, 512], F32, kind="ExternalOutput")
    with tile.TileContext(nc) as tc:
        with tc.tile_pool(name="sb", bufs=2) as pool:
            t = pool.tile([128, 512], F32)
            nc.sync.dma_start(t[:], x[:])
            t2 = pool.tile([128, 512], F32)
            nc.scalar.mul(t2[:], t[:], 2.0)
            nc.sync.dma_start(y[:], t2[:])
    nc.finalize()
    im = [{"x": np.zeros((128, 512), np.float32)} for _ in range(NC_)]
    return nc, im


def bench_ns(inputs, pairs=40):
    import time
    import jax
    in_maps, meta = _prep(**inputs)
    nc = _build(meta)
    run_k, args_k = _make_runner(nc, in_maps)
    tnc, tim = _trivial_nc()
    run_t, args_t = _make_runner(tnc, tim)
    jax.block_until_ready(run_k(*args_k))
    jax.block_until_ready(run_t(*args_t))
    dk, dt = [], []
    for _ in range(pairs):
        t0 = time.perf_counter()
        jax.block_until_ready(run_t(*args_t))
        t1 = time.perf_counter()
        jax.block_until_ready(run_k(*args_k))
        t2 = time.perf_counter()
        dt.append(t1 - t0)
        dk.append(t2 - t1)
    dk, dt = np.array(dk), np.array(dt)
    est = np.median(dk) - np.median(dt)
    est_min = dk.min() - dt.min()
    return int(est * 1e9), int(est_min * 1e9)
